# revision 11
# baseline (speedup 1.0000x reference)
"""Trainium2 Bass kernel for nn_Block_47880295416554 (windowed-attention
transformer block with RoPE, EVA/Swin style).

Sharding: data-parallel over batch B=8 across the 8 NeuronCores; weights
replicated. Each core runs the full block on one [64, 64, 768] image.

Per-core pipeline (all matmuls in bf16, fp32 accumulate; residual spine fp32):
  A: LN1 (g/b folded into qkv weights/bias) -> PE-transpose -> q,k feature-
     major + RoPE -> padded-grid scratch; v token-major -> window scratch.
  B: per (window, head): scoresT = k_win^T q_win on PE, exp on ACT (no max
     subtraction; scores are small), out = v_aug^T @ expT with a ones column
     giving the softmax denominator; normalize; store feature-major.
  C: proj (gamma_1 folded into weights, b_v folded into bias) + residual,
     LN2, transpose.
  D: MLP fc1+gelu (feature-major) then fc2 (gamma_2 folded) + residual.
"""
import numpy as np

C = 768
G = 64          # grid H = W
GP = 70         # padded grid (5 windows of 14)
WS = 14
NW1 = 5
NW = 25
T = 196         # tokens per window
NH = 12
HD = 64
HF = 3072
TOK = 4096
CH = 512        # token chunk
NCH = 8
CC = 6          # C / 128
LN_EPS = 1e-6
KC0, KC1 = 126, 70   # window token chunks (9 rows, 5 rows)

_COMPILED = None


def _build(sim_gelu=False):
    import concourse.bacc as bacc
    import concourse.mybir as mybir
    from concourse import tile, masks
    from concourse.alu_op_type import AluOpType

    F32 = mybir.dt.float32
    BF16 = mybir.dt.bfloat16
    AF = mybir.ActivationFunctionType

    nc = bacc.Bacc(None, target_bir_lowering=False, debug=False)

    # ---- I/O ----
    x_in = nc.declare_dram_parameter("x", [G, G, C], F32, isOutput=False)
    rope_in = nc.declare_dram_parameter("rope_2d", [G, G, HD], F32, isOutput=False)
    ln1_g = nc.declare_dram_parameter("ln1_g", [C], F32, isOutput=False)
    ln1_b = nc.declare_dram_parameter("ln1_b", [C], F32, isOutput=False)
    w_qkv = nc.declare_dram_parameter("w_qkv", [3 * C, C], F32, isOutput=False)
    b_qkv = nc.declare_dram_parameter("b_qkv", [3 * C], F32, isOutput=False)
    w_proj = nc.declare_dram_parameter("w_proj", [C, C], F32, isOutput=False)
    b_proj = nc.declare_dram_parameter("b_proj", [C], F32, isOutput=False)
    gamma_1 = nc.declare_dram_parameter("gamma_1", [C], F32, isOutput=False)
    ln2_g = nc.declare_dram_parameter("ln2_g", [C], F32, isOutput=False)
    ln2_b = nc.declare_dram_parameter("ln2_b", [C], F32, isOutput=False)
    w_fc1 = nc.declare_dram_parameter("w_fc1", [HF, C], F32, isOutput=False)
    b_fc1 = nc.declare_dram_parameter("b_fc1", [HF], F32, isOutput=False)
    w_fc2 = nc.declare_dram_parameter("w_fc2", [C, HF], F32, isOutput=False)
    b_fc2 = nc.declare_dram_parameter("b_fc2", [C], F32, isOutput=False)
    gamma_2 = nc.declare_dram_parameter("gamma_2", [C], F32, isOutput=False)
    out = nc.declare_dram_parameter("out", [G, G, C], F32, isOutput=True)

    # ---- DRAM scratch ----
    q_dram = nc.dram_tensor("q_dram", [NH, HD, GP, GP], BF16)
    k_dram = nc.dram_tensor("k_dram", [NH, HD, GP, GP], BF16)
    v_dram = nc.dram_tensor("v_dram", [NW, T, C], BF16)
    att_dram = nc.dram_tensor("att_dram", [C, GP, GP], BF16)
    x1_dram = nc.dram_tensor("x1_dram", [TOK, C], F32)

    x_t = x_in.rearrange("r c d -> (r c) d")      # [4096, 768]
    out_t = out.rearrange("r c d -> (r c) d")


    with tile.TileContext(nc) as tc:
        with (
            tc.tile_pool(name="const", bufs=1) as const,
            tc.tile_pool(name="wpool", bufs=1) as wpool,
        ):
            wqk = tc.alloc_tile_pool(name="wqk", bufs=1)
            # ============ PREP ============
            ident = const.tile([128, 128], BF16)
            masks.make_identity(nc, ident[:])
            zeros_bf = const.tile([128, 768], BF16)
            nc.vector.memset(zeros_bf[:], 0.0)

            # gamma rows and bias rows
            g1_row = const.tile([1, C], F32)
            nc.sync.dma_start(g1_row[:], gamma_1[None, :])
            g2_row = const.tile([1, C], F32)
            nc.sync.dma_start(g2_row[:], gamma_2[None, :])
            bproj_row = const.tile([1, C], F32)
            nc.sync.dma_start(bproj_row[:], b_proj[None, :])
            bfc2_row = const.tile([1, C], F32)
            nc.sync.dma_start(bfc2_row[:], b_fc2[None, :])

            # ln gains as [128, CC] feature-major (per-partition scalars)
            ln1g_fm = const.tile([128, CC], F32)
            nc.sync.dma_start(ln1g_fm[:], ln1_g.rearrange("(a p) -> p a", p=128))
            ln2g_fm = const.tile([128, CC], F32)
            nc.sync.dma_start(ln2g_fm[:], ln2_g.rearrange("(a p) -> p a", p=128))
            ln1b_fm = const.tile([128, CC], F32)
            nc.sync.dma_start(ln1b_fm[:], ln1_b.rearrange("(a p) -> p a", p=128))
            ln2b_fm = const.tile([128, CC], F32)
            nc.sync.dma_start(ln2b_fm[:], ln2_b.rearrange("(a p) -> p a", p=128))

            sincos = wqk.tile([128, 2 * TOK], BF16)  # [:, :TOK]=SIN, [:, TOK:]=COS
            SIN = sincos[:, 0:TOK]
            COS = sincos[:, TOK:2 * TOK]

            with (
                tc.tile_pool(name="prep_sb", bufs=1) as prep_sb,
                tc.tile_pool(name="prep_ps", bufs=1, space="PSUM") as prep_ps,
            ):
                # COS/SIN feature-major [128, 4096] bf16 (4 head-replicas of 32)
                cs_f = prep_sb.tile([64, TOK], F32, name="cs_f")
                nc.sync.dma_start(
                    cs_f[:],
                    rope_in.rearrange("r c d -> d (r c)"))
                for k in range(4):
                    nc.vector.tensor_copy(sincos[32 * k:32 * (k + 1), 0:TOK],
                                          cs_f[0:32, :])
                    nc.vector.tensor_copy(sincos[32 * k:32 * (k + 1), TOK:2 * TOK],
                                          cs_f[32:64, :])
                # gamma bcast tiles (for free-dim weight folds)
                g1b = prep_sb.tile([128, C], F32)
                nc.gpsimd.partition_broadcast(g1b[:], g1_row[:])
                g2b = prep_sb.tile([128, C], F32)
                nc.gpsimd.partition_broadcast(g2b[:], g2_row[:])

                # ---- weights: WqkT (permuted q,k + natural v), ln1_g fold ----
                wqkT = [wqk.tile([128, 3 * C], BF16, name=f"wqkT{c}")
                        for c in range(CC)]
                bias_ps = prep_ps.tile([128, 18], F32)
                lnb1_fm = [ln1b_fm[:, c:c + 1] for c in range(CC)]
                for j in range(18):
                    for c in range(CC):
                        raw = prep_sb.tile([128, 128], F32, tag="rawW", bufs=3,
                                         name=f"raw_{c}_{j}")
                        if j < 12:
                            sect, jj = divmod(j, 6)
                            half = jj // 3
                            h0 = 4 * (jj % 3)
                            base = sect * C + h0 * HD + half * 32
                            for hh in range(4):
                                r0_ = base + hh * HD
                                nc.sync.dma_start(
                                    raw[:, hh * 32:(hh + 1) * 32],
                                    w_qkv[r0_:r0_ + 32, c * 128:(c + 1) * 128]
                                    .rearrange("d c -> c d"))
                        else:
                            f0 = 2 * C + (j - 12) * 128
                            nc.sync.dma_start(
                                raw[:],
                                w_qkv[f0:f0 + 128, c * 128:(c + 1) * 128]
                                .rearrange("f c -> c f"))
                        # bias contribution: raw.T @ ln1_b  -> [128 f, 1]
                        nc.tensor.matmul(bias_ps[:, j:j + 1], raw[:], lnb1_fm[c],
                                         start=(c == 0), stop=(c == CC - 1))
                        # fold ln1_g (per-partition = per-C-row) and cast
                        nc.vector.tensor_scalar(
                            wqkT[c][:, j * 128:(j + 1) * 128], raw[:],
                            ln1g_fm[:, c:c + 1], None, op0=AluOpType.mult)
                bias_qkv_fm = wqk.tile([128, 18], F32)
                # b_qkv in permuted feature-major order
                bq_perm = wqk.tile([128, 18], F32)
                for j in range(12):
                    sect, jj = divmod(j, 6)
                    half = jj // 3
                    h0 = 4 * (jj % 3)
                    base = sect * C + h0 * HD + half * 32
                    for hh in range(4):
                        r0_ = base + hh * HD
                        nc.sync.dma_start(
                            bq_perm[hh * 32:(hh + 1) * 32, j:j + 1],
                            b_qkv[r0_:r0_ + 32, None])
                for j in range(12, 18):
                    f0 = 2 * C + (j - 12) * 128
                    nc.sync.dma_start(bq_perm[:, j:j + 1],
                                      b_qkv[f0:f0 + 128, None])
                nc.vector.tensor_tensor(bias_qkv_fm[:], bias_ps[:], bq_perm[:],
                                        op=AluOpType.add)

                # ---- W1T with ln2_g fold + bias ----
                w1T = [wpool.tile([128, HF], BF16, name=f"w1T{c}")
                       for c in range(CC)]
                bias1_ps = prep_ps.tile([128, 24], F32)
                for j in range(24):
                    for c in range(CC):
                        raw = prep_sb.tile([128, 128], F32, tag="rawW", bufs=3,
                                         name=f"raw1_{c}_{j}")
                        nc.sync.dma_start(
                            raw[:],
                            w_fc1[j * 128:(j + 1) * 128, c * 128:(c + 1) * 128]
                            .rearrange("f c -> c f"))
                        nc.tensor.matmul(bias1_ps[:, j:j + 1], raw[:],
                                         ln2b_fm[:, c:c + 1],
                                         start=(c == 0), stop=(c == CC - 1))
                        nc.vector.tensor_scalar(
                            w1T[c][:, j * 128:(j + 1) * 128], raw[:],
                            ln2g_fm[:, c:c + 1], None, op0=AluOpType.mult)
                bias_fc1_fm = const.tile([128, 24], F32)
                b1_fm = const.tile([128, 24], F32)
                nc.sync.dma_start(b1_fm[:], b_fc1.rearrange("(a p) -> p a", p=128))
                nc.vector.tensor_tensor(bias_fc1_fm[:], bias1_ps[:], b1_fm[:],
                                        op=AluOpType.add)

                # ---- WpT_eff (gamma_1 row fold) + bp_eff row ----
                wpT = [wpool.tile([128, C], BF16, name=f"wpT{c}")
                       for c in range(CC)]
                bv_fm = const.tile([128, CC], F32)
                nc.sync.dma_start(bv_fm[:],
                                  b_qkv[2 * C:3 * C].rearrange("(a p) -> p a", p=128))
                brow_ps = prep_ps.tile([1, C], F32)
                for c in range(CC):
                    rawp = prep_sb.tile([128, C], F32, tag="rawWp", bufs=2,
                                      name=f"rawp_{c}")
                    nc.sync.dma_start(rawp[:],
                                      w_proj[:, c * 128:(c + 1) * 128]
                                      .rearrange("f c -> c f"))
                    # Wp @ b_v row: lhsT = b_v_fm chunk [128,1], rhs = rawp
                    nc.tensor.matmul(brow_ps[:, 0:512], bv_fm[:, c:c + 1],
                                     rawp[:, 0:512],
                                     start=(c == 0), stop=(c == CC - 1))
                    nc.tensor.matmul(brow_ps[:, 512:768], bv_fm[:, c:c + 1],
                                     rawp[:, 512:768],
                                     start=(c == 0), stop=(c == CC - 1))
                    nc.vector.tensor_tensor(wpT[c][:], rawp[:], g1b[:],
                                            op=AluOpType.mult)
                bp_row = const.tile([1, C], F32)
                nc.vector.tensor_tensor(bp_row[:], brow_ps[:], bproj_row[:],
                                        op=AluOpType.add)
                nc.vector.tensor_tensor(bp_row[:], bp_row[:], g1_row[:],
                                        op=AluOpType.mult)
                bp_bcast = const.tile([128, C], F32)
                nc.gpsimd.partition_broadcast(bp_bcast[:], bp_row[:])

                # ---- W2T_eff (gamma_2 row fold) + b2 row ----
                w2T = [wpool.tile([128, C], BF16, name=f"w2T{j}")
                       for j in range(24)]
                for j in range(24):
                    raw2 = prep_sb.tile([128, C], F32, tag="rawWp", bufs=2,
                                      name=f"raw2_{j}")
                    nc.sync.dma_start(raw2[:],
                                      w_fc2[:, j * 128:(j + 1) * 128]
                                      .rearrange("f c -> c f"))
                    nc.vector.tensor_tensor(w2T[j][:], raw2[:], g2b[:],
                                            op=AluOpType.mult)
                b2_row = const.tile([1, C], F32)
                nc.vector.tensor_tensor(b2_row[:], bfc2_row[:], g2_row[:],
                                        op=AluOpType.mult)
                b2_bcast = const.tile([128, C], F32)
                nc.gpsimd.partition_broadcast(b2_bcast[:], b2_row[:])

            # ---- zero pad strips of q_dram/k_dram, pad tokens of v_dram ----
            for dram in (q_dram, k_dram):
                for h in range(NH):
                    nc.sync.dma_start(
                        dram[h, :, :, G:GP],
                        zeros_bf[0:HD, 0:GP * 6].rearrange("d (r c) -> d r c", c=6))
                    nc.sync.dma_start(
                        dram[h, :, G:GP, 0:G],
                        zeros_bf[0:HD, 0:6 * G].rearrange("d (r c) -> d r c", c=G))
            for wr in range(NW1):
                for wc in range(NW1):
                    w = wr * NW1 + wc
                    if wc == NW1 - 1:
                        # cols 8..13 of every row are padding
                        nc.sync.dma_start(
                            v_dram[w].rearrange("(r c) f -> r c f", c=WS)[:, 8:WS, :],
                            zeros_bf[0:84, :])
                    if wr == NW1 - 1:
                        # rows 8..13 (tokens 112:196) are padding
                        nc.sync.dma_start(v_dram[w, 112:T, :], zeros_bf[0:84, :])

            # ============ PHASE A ============
            with (
                tc.tile_pool(name="a_sb", bufs=2) as a_sb,
                tc.tile_pool(name="a_ps", bufs=2, space="PSUM") as a_ps,
                tc.tile_pool(name="a_tr_ps", bufs=2, space="PSUM") as a_tr_ps,
                tc.tile_pool(name="a_vps", bufs=2, space="PSUM") as a_vps,
            ):
                for ch in range(NCH):
                    xlnT = [a_sb.tile([128, CH], BF16, tag=f"xlnT{c}",
                                      name=f"xlnT_{ch}_{c}") for c in range(CC)]
                    for t4 in range(4):
                        tok0 = ch * CH + t4 * 128
                        xt = a_sb.tile([128, C], F32, tag="xt", bufs=2,
                                       name=f"xt_{ch}_{t4}")
                        nc.sync.dma_start(xt[:], x_t[tok0:tok0 + 128, :])
                        stats = a_sb.tile([128, 12], F32, tag="stats", bufs=3,
                                          name=f"st_{ch}_{t4}")
                        nc.vector.bn_stats(stats[:, 0:6], xt[:, 0:384])
                        nc.vector.bn_stats(stats[:, 6:12], xt[:, 384:768])
                        mv = a_sb.tile([128, 2], F32, tag="mv", bufs=3,
                                       name=f"mv_{ch}_{t4}")
                        nc.vector.bn_aggr(mv[:], stats[:])
                        rs = a_sb.tile([128, 1], F32, tag="rs", bufs=3,
                                       name=f"rs_{ch}_{t4}")
                        nc.vector.tensor_scalar_add(rs[:], mv[:, 1:2], LN_EPS)
                        nc.vector.reciprocal(rs[:], rs[:])
                        nc.scalar.sqrt(rs[:], rs[:])
                        xln = a_sb.tile([128, C], BF16, tag="xln", bufs=3,
                                        name=f"xln_{ch}_{t4}")
                        nc.vector.tensor_scalar(xln[:], xt[:], mv[:, 0:1], rs[:],
                                                op0=AluOpType.subtract,
                                                op1=AluOpType.mult)
                        for c in range(CC):
                            trp = a_tr_ps.tile([128, 128], BF16, tag="trp",
                                               name=f"trp_{ch}_{t4}_{c}")
                            nc.tensor.transpose(trp[:], xln[:, c * 128:(c + 1) * 128],
                                                ident[:])
                            nc.scalar.copy(xlnT[c][:, t4 * 128:(t4 + 1) * 128],
                                           trp[:])
                    # q,k feature-major with rope (pairwise to bound slots)
                    cosc = COS[:, ch * CH:(ch + 1) * CH]
                    sinc = SIN[:, ch * CH:(ch + 1) * CH]
                    for sect in range(2):
                        dram = q_dram if sect == 0 else k_dram
                        for jp in range(3):
                            pair = []
                            for j in (sect * 6 + jp, sect * 6 + jp + 3):
                                ps = a_ps.tile([128, CH], F32, tag="qkps",
                                               name=f"qkps_{ch}_{j}")
                                for c in range(CC):
                                    nc.tensor.matmul(
                                        ps[:], wqkT[c][:, j * 128:(j + 1) * 128],
                                        xlnT[c][:], start=(c == 0),
                                        stop=(c == CC - 1))
                                sb = a_sb.tile([128, CH], F32, tag="qksb", bufs=4,
                                               name=f"qksb_{ch}_{j}")
                                nc.scalar.activation(sb[:], ps[:], AF.Identity,
                                                     bias=bias_qkv_fm[:, j:j + 1])
                                pair.append(sb)
                            q1, q2 = pair
                            t1 = a_sb.tile([128, CH], F32, tag="ropet1", bufs=2,
                                           name=f"t1_{ch}_{sect}_{jp}")
                            t2 = a_sb.tile([128, CH], F32, tag="ropet2", bufs=2,
                                           name=f"t2_{ch}_{sect}_{jp}")
                            r1 = a_sb.tile([128, CH], BF16, tag="roper1", bufs=2,
                                           name=f"r1_{ch}_{sect}_{jp}")
                            r2 = a_sb.tile([128, CH], BF16, tag="roper2", bufs=2,
                                           name=f"r2_{ch}_{sect}_{jp}")
                            nc.vector.tensor_tensor(t1[:], q1[:], cosc,
                                                    op=AluOpType.mult)
                            nc.vector.tensor_tensor(t2[:], q2[:], sinc,
                                                    op=AluOpType.mult)
                            nc.vector.tensor_tensor(r1[:], t1[:], t2[:],
                                                    op=AluOpType.subtract)
                            nc.vector.tensor_tensor(t1[:], q2[:], cosc,
                                                    op=AluOpType.mult)
                            nc.vector.tensor_tensor(t2[:], q1[:], sinc,
                                                    op=AluOpType.mult)
                            nc.vector.tensor_tensor(r2[:], t1[:], t2[:],
                                                    op=AluOpType.add)
                            # store: rows of r1 are heads 4jp..4jp+3, d 0..31
                            r0 = ch * 8  # 8 grid rows per chunk
                            for hh in range(4):
                                h = 4 * jp + hh
                                nc.sync.dma_start(
                                    dram[h, 0:32, r0:r0 + 8, 0:G],
                                    r1[32 * hh:32 * (hh + 1), :]
                                    .rearrange("d (r c) -> d r c", c=G))
                                nc.sync.dma_start(
                                    dram[h, 32:64, r0:r0 + 8, 0:G],
                                    r2[32 * hh:32 * (hh + 1), :]
                                    .rearrange("d (r c) -> d r c", c=G))
                    # v token-major
                    for t4 in range(4):
                        tok0 = ch * CH + t4 * 128
                        vbf = a_sb.tile([128, C], BF16, tag="vbf", bufs=2,
                                        name=f"vbf_{ch}_{t4}")
                        for nchunk, (f0, fn) in enumerate(((0, 512), (512, 256))):
                            vps = a_vps.tile([128, 512], F32, tag="vps",
                                             name=f"vps_{ch}_{t4}_{nchunk}")
                            for c in range(CC):
                                nc.tensor.matmul(
                                    vps[:, 0:fn],
                                    xlnT[c][:, t4 * 128:(t4 + 1) * 128],
                                    wqkT[c][:, 2 * C + f0:2 * C + f0 + fn],
                                    start=(c == 0), stop=(c == CC - 1))
                            nc.scalar.copy(vbf[:, f0:f0 + fn], vps[:, 0:fn])
                        # scatter rows into window scratch
                        gr0 = (ch * CH + t4 * 128) // G  # first grid row
                        for rr in range(2):
                            r = gr0 + rr
                            wr, r_in = divmod(r, WS)
                            for wc in range(NW1):
                                w = wr * NW1 + wc
                                cw = 8 if wc == NW1 - 1 else WS
                                nc.sync.dma_start(
                                    v_dram[w, r_in * WS:r_in * WS + cw, :],
                                    vbf[rr * 64 + wc * WS:rr * 64 + wc * WS + cw, :])

            wqk.release()

            # ============ PHASE B: attention ============
            with (
                tc.tile_pool(name="b_sb", bufs=3) as b_sb,
                tc.tile_pool(name="b_ps", bufs=4, space="PSUM") as b_ps,
                tc.tile_pool(name="b_ops", bufs=2, space="PSUM") as b_ops,
            ):
                for w in range(NW):
                    wr, wc = divmod(w, NW1)
                    r0, c0 = wr * WS, wc * WS
                    for h in range(NH):
                        qw = b_sb.tile([HD, T], BF16, tag="qw",
                                       name=f"qw_{w}_{h}")
                        kw = b_sb.tile([HD, T], BF16, tag="kw",
                                       name=f"kw_{w}_{h}")
                        nc.sync.dma_start(
                            qw[:].rearrange("d (r c) -> d r c", c=WS),
                            q_dram[h, :, r0:r0 + WS, c0:c0 + WS])
                        nc.sync.dma_start(
                            kw[:].rearrange("d (r c) -> d r c", c=WS),
                            k_dram[h, :, r0:r0 + WS, c0:c0 + WS])
                        v0 = b_sb.tile([KC0, 65], BF16, tag="v0",
                                       name=f"v0_{w}_{h}")
                        v1 = b_sb.tile([KC1, 65], BF16, tag="v1",
                                       name=f"v1_{w}_{h}")
                        nc.sync.dma_start(v0[:, 0:64],
                                          v_dram[w, 0:KC0, h * HD:(h + 1) * HD])
                        nc.sync.dma_start(v1[:, 0:64],
                                          v_dram[w, KC0:T, h * HD:(h + 1) * HD])
                        nc.vector.memset(v0[:, 64:65], 1.0)
                        nc.vector.memset(v1[:, 64:65], 1.0)
                        s0 = b_ps.tile([KC0, T], F32, tag="s", name=f"s0_{w}_{h}")
                        s1 = b_ps.tile([KC1, T], F32, tag="s", name=f"s1_{w}_{h}")
                        nc.tensor.matmul(s0[:], kw[:, 0:KC0], qw[:],
                                         start=True, stop=True)
                        nc.tensor.matmul(s1[:], kw[:, KC0:T], qw[:],
                                         start=True, stop=True)
                        e0 = b_sb.tile([KC0, T], BF16, tag="e0",
                                       name=f"e0_{w}_{h}")
                        e1 = b_sb.tile([KC1, T], BF16, tag="e1",
                                       name=f"e1_{w}_{h}")
                        nc.scalar.activation(e0[:], s0[:], AF.Exp, scale=HD ** -0.5)
                        nc.scalar.activation(e1[:], s1[:], AF.Exp, scale=HD ** -0.5)
                        o = b_ops.tile([65, T], F32, tag="o", name=f"o_{w}_{h}")
                        nc.tensor.matmul(o[:], v0[:], e0[:], start=True, stop=False)
                        nc.tensor.matmul(o[:], v1[:], e1[:], start=False, stop=True)
                        rcp = b_sb.tile([1, T], F32, tag="rcp", name=f"rcp_{w}_{h}")
                        nc.vector.reciprocal(rcp[:], o[64:65, :])
                        rb = b_sb.tile([HD, T], F32, tag="rb", name=f"rb_{w}_{h}")
                        nc.gpsimd.partition_broadcast(rb[:], rcp[:])
                        ab = b_sb.tile([HD, T], BF16, tag="ab", name=f"ab_{w}_{h}")
                        nc.vector.tensor_tensor(ab[:], o[0:64, :], rb[:],
                                                op=AluOpType.mult)
                        nc.sync.dma_start(
                            att_dram[h * HD:(h + 1) * HD, r0:r0 + WS, c0:c0 + WS],
                            ab[:].rearrange("d (r c) -> d r c", c=WS))

            # ============ PHASE C: proj + residual + LN2 + transpose ============
            with (
                tc.tile_pool(name="c_sb", bufs=2) as c_sb,
                tc.tile_pool(name="c_ps", bufs=2, space="PSUM") as c_ps,
                tc.tile_pool(name="c_tr_ps", bufs=2, space="PSUM") as c_tr_ps,
            ):
                for ch in range(NCH):
                    yT = [c_sb.tile([128, CH], BF16, tag=f"yT{c}",
                                    name=f"yT_{ch}_{c}") for c in range(CC)]
                    for t4 in range(4):
                        tok0 = ch * CH + t4 * 128
                        gr0 = tok0 // G
                        attT = []
                        for c in range(CC):
                            at = c_sb.tile([128, 128], BF16, tag=f"attT{c}", bufs=2,
                                           name=f"attT_{ch}_{t4}_{c}")
                            nc.sync.dma_start(
                                at[:].rearrange("f (r c) -> f r c", c=G),
                                att_dram[c * 128:(c + 1) * 128, gr0:gr0 + 2, 0:G])
                            attT.append(at)
                        x1 = c_sb.tile([128, C], F32, tag="x1", bufs=2,
                                       name=f"x1_{ch}_{t4}")
                        xt = c_sb.tile([128, C], F32, tag="xt2", bufs=2,
                                       name=f"xt2_{ch}_{t4}")
                        nc.sync.dma_start(xt[:], x_t[tok0:tok0 + 128, :])
                        for f0, fn in ((0, 512), (512, 256)):
                            pps = c_ps.tile([128, 512], F32, tag="pps",
                                            name=f"pps_{ch}_{t4}_{f0}")
                            for c in range(CC):
                                nc.tensor.matmul(pps[:, 0:fn], attT[c][:],
                                                 wpT[c][:, f0:f0 + fn],
                                                 start=(c == 0), stop=(c == CC - 1))
                            nc.vector.scalar_tensor_tensor(
                                x1[:, f0:f0 + fn], pps[:, 0:fn], 1.0,
                                bp_bcast[:, f0:f0 + fn],
                                op0=AluOpType.mult, op1=AluOpType.add)
                        nc.vector.tensor_tensor(x1[:], x1[:], xt[:],
                                                op=AluOpType.add)
                        nc.sync.dma_start(x1_dram[tok0:tok0 + 128, :], x1[:])
                        # LN2
                        stats = c_sb.tile([128, 12], F32, tag="stats2", bufs=3,
                                          name=f"st2_{ch}_{t4}")
                        nc.vector.bn_stats(stats[:, 0:6], x1[:, 0:384])
                        nc.vector.bn_stats(stats[:, 6:12], x1[:, 384:768])
                        mv = c_sb.tile([128, 2], F32, tag="mv2", bufs=3,
                                       name=f"mv2_{ch}_{t4}")
                        nc.vector.bn_aggr(mv[:], stats[:])
                        rs = c_sb.tile([128, 1], F32, tag="rs2", bufs=3,
                                       name=f"rs2_{ch}_{t4}")
                        nc.vector.tensor_scalar_add(rs[:], mv[:, 1:2], LN_EPS)
                        nc.vector.reciprocal(rs[:], rs[:])
                        nc.scalar.sqrt(rs[:], rs[:])
                        y = c_sb.tile([128, C], BF16, tag="y", bufs=3,
                                      name=f"y_{ch}_{t4}")
                        nc.vector.tensor_scalar(y[:], x1[:], mv[:, 0:1], rs[:],
                                                op0=AluOpType.subtract,
                                                op1=AluOpType.mult)
                        for c in range(CC):
                            trp = c_tr_ps.tile([128, 128], BF16, tag="trp2",
                                               name=f"trp2_{ch}_{t4}_{c}")
                            nc.tensor.transpose(trp[:], y[:, c * 128:(c + 1) * 128],
                                                ident[:])
                            nc.scalar.copy(yT[c][:, t4 * 128:(t4 + 1) * 128],
                                           trp[:])
                    # ============ PHASE D (per chunk): MLP ============
                    h1 = [c_sb.tile([128, CH], BF16, tag=f"h1_{j}", bufs=1,
                                    name=f"h1_{ch}_{j}") for j in range(24)]
                    for j in range(24):
                        hps = c_ps.tile([128, 512], F32, tag="hps",
                                        name=f"hps_{ch}_{j}")
                        for c in range(CC):
                            nc.tensor.matmul(hps[:], w1T[c][:, j * 128:(j + 1) * 128],
                                             yT[c][:], start=(c == 0),
                                             stop=(c == CC - 1))
                        if not sim_gelu:
                            nc.scalar.activation(h1[j][:], hps[:], AF.Gelu,
                                                 bias=bias_fc1_fm[:, j:j + 1])
                        else:
                            # CoreSim lacks Gelu: tanh-approx decomposition
                            tg = c_sb.tile([128, CH], F32, tag="tg", bufs=2,
                                           name=f"tg_{ch}_{j}")
                            nc.scalar.activation(tg[:], hps[:], AF.Identity,
                                                 bias=bias_fc1_fm[:, j:j + 1])
                            sq = c_sb.tile([128, CH], F32, tag="sq", bufs=2,
                                           name=f"sq_{ch}_{j}")
                            nc.scalar.activation(sq[:], tg[:], AF.Square)
                            nc.vector.tensor_scalar(sq[:], sq[:], 0.044715, 1.0,
                                                    op0=AluOpType.mult,
                                                    op1=AluOpType.add)
                            nc.vector.tensor_tensor(sq[:], sq[:], tg[:],
                                                    op=AluOpType.mult)
                            nc.scalar.activation(sq[:], sq[:], AF.Tanh,
                                                 scale=0.7978845608028654)
                            nc.vector.tensor_scalar(sq[:], sq[:], 1.0, 0.5,
                                                    op0=AluOpType.add,
                                                    op1=AluOpType.mult)
                            nc.vector.tensor_tensor(h1[j][:], sq[:], tg[:],
                                                    op=AluOpType.mult)
                    for t4 in range(4):
                        tok0 = ch * CH + t4 * 128
                        x1t = c_sb.tile([128, C], F32, tag="x1t", bufs=2,
                                        name=f"x1t_{ch}_{t4}")
                        nc.sync.dma_start(x1t[:], x1_dram[tok0:tok0 + 128, :])
                        ot = c_sb.tile([128, C], F32, tag="ot", bufs=2,
                                       name=f"ot_{ch}_{t4}")
                        for f0, fn in ((0, 512), (512, 256)):
                            ops_ = c_ps.tile([128, 512], F32, tag="ops",
                                             name=f"ops_{ch}_{t4}_{f0}")
                            for j in range(24):
                                nc.tensor.matmul(
                                    ops_[:, 0:fn],
                                    h1[j][:, t4 * 128:(t4 + 1) * 128],
                                    w2T[j][:, f0:f0 + fn],
                                    start=(j == 0), stop=(j == 23))
                            nc.vector.scalar_tensor_tensor(
                                ot[:, f0:f0 + fn], ops_[:, 0:fn], 1.0,
                                b2_bcast[:, f0:f0 + fn],
                                op0=AluOpType.mult, op1=AluOpType.add)
                        nc.vector.tensor_tensor(ot[:], ot[:], x1t[:],
                                                op=AluOpType.add)
                        nc.sync.dma_start(out_t[tok0:tok0 + 128, :], ot[:])

    nc.finalize()
    return nc


def kernel(**inputs) -> np.ndarray:
    global _COMPILED
    from concourse.bass_utils import run_bass_kernel_spmd

    if _COMPILED is None:
        _COMPILED = _build()
    nc = _COMPILED

    x = np.ascontiguousarray(np.asarray(inputs["x"], dtype=np.float32))
    rope = np.ascontiguousarray(
        np.asarray(inputs["rope_2d"], dtype=np.float32).reshape(G, G, HD))
    shared = {
        k: np.ascontiguousarray(np.asarray(inputs[k], dtype=np.float32))
        for k in ("ln1_g", "ln1_b", "w_qkv", "b_qkv", "w_proj", "b_proj",
                  "gamma_1", "ln2_g", "ln2_b", "w_fc1", "b_fc1", "w_fc2",
                  "b_fc2", "gamma_2")
    }
    in_maps = [{"x": x[b], "rope_2d": rope, **shared} for b in range(8)]
    res = run_bass_kernel_spmd(nc, in_maps, list(range(8)))
    return np.stack([res.results[b]["out"] for b in range(8)]).astype(np.float32)


# revision 14
# speedup vs baseline: 1.0020x; 1.0020x over previous
"""Trainium2 Bass kernel for nn_Block_47880295416554 (windowed-attention
transformer block with RoPE, EVA/Swin style).

Sharding: data-parallel over batch B=8 across the 8 NeuronCores; weights
replicated. Each core runs the full block on one [64, 64, 768] image.

Per-core pipeline (all matmuls in bf16, fp32 accumulate; residual spine fp32):
  A: LN1 (g/b folded into qkv weights/bias) -> PE-transpose -> q,k feature-
     major + RoPE -> padded-grid scratch; v token-major -> window scratch.
  B: per (window, head): scoresT = k_win^T q_win on PE, exp on ACT (no max
     subtraction; scores are small), out = v_aug^T @ expT with a ones column
     giving the softmax denominator; normalize; store feature-major.
  C: proj (gamma_1 folded into weights, b_v folded into bias) + residual,
     LN2, transpose.
  D: MLP fc1+gelu (feature-major) then fc2 (gamma_2 folded) + residual.
"""
import numpy as np

C = 768
G = 64          # grid H = W
GP = 70         # padded grid (5 windows of 14)
WS = 14
NW1 = 5
NW = 25
T = 196         # tokens per window
NH = 12
HD = 64
HF = 3072
TOK = 4096
CH = 512        # token chunk
NCH = 8
CC = 6          # C / 128
LN_EPS = 1e-6
KC0, KC1 = 126, 70   # window token chunks (9 rows, 5 rows)

_COMPILED = None


def _build(sim_gelu=False):
    import concourse.bacc as bacc
    import concourse.mybir as mybir
    from concourse import tile, masks
    from concourse.alu_op_type import AluOpType

    F32 = mybir.dt.float32
    BF16 = mybir.dt.bfloat16
    AF = mybir.ActivationFunctionType

    nc = bacc.Bacc(None, target_bir_lowering=False, debug=False)

    # ---- I/O ----
    x_in = nc.declare_dram_parameter("x", [G, G, C], F32, isOutput=False)
    rope_in = nc.declare_dram_parameter("rope_2d", [G, G, HD], F32, isOutput=False)
    ln1_g = nc.declare_dram_parameter("ln1_g", [C], F32, isOutput=False)
    ln1_b = nc.declare_dram_parameter("ln1_b", [C], F32, isOutput=False)
    w_qkv = nc.declare_dram_parameter("w_qkv", [3 * C, C], F32, isOutput=False)
    b_qkv = nc.declare_dram_parameter("b_qkv", [3 * C], F32, isOutput=False)
    w_proj = nc.declare_dram_parameter("w_proj", [C, C], F32, isOutput=False)
    b_proj = nc.declare_dram_parameter("b_proj", [C], F32, isOutput=False)
    gamma_1 = nc.declare_dram_parameter("gamma_1", [C], F32, isOutput=False)
    ln2_g = nc.declare_dram_parameter("ln2_g", [C], F32, isOutput=False)
    ln2_b = nc.declare_dram_parameter("ln2_b", [C], F32, isOutput=False)
    w_fc1 = nc.declare_dram_parameter("w_fc1", [HF, C], F32, isOutput=False)
    b_fc1 = nc.declare_dram_parameter("b_fc1", [HF], F32, isOutput=False)
    w_fc2 = nc.declare_dram_parameter("w_fc2", [C, HF], F32, isOutput=False)
    b_fc2 = nc.declare_dram_parameter("b_fc2", [C], F32, isOutput=False)
    gamma_2 = nc.declare_dram_parameter("gamma_2", [C], F32, isOutput=False)
    out = nc.declare_dram_parameter("out", [G, G, C], F32, isOutput=True)

    # ---- DRAM scratch ----
    q_dram = nc.dram_tensor("q_dram", [NH, HD, GP, GP], BF16)
    k_dram = nc.dram_tensor("k_dram", [NH, HD, GP, GP], BF16)
    v_dram = nc.dram_tensor("v_dram", [NW, T, C], BF16)
    att_dram = nc.dram_tensor("att_dram", [C, GP, GP], BF16)
    x1_dram = nc.dram_tensor("x1_dram", [TOK, C], F32)

    x_t = x_in.rearrange("r c d -> (r c) d")      # [4096, 768]
    out_t = out.rearrange("r c d -> (r c) d")


    with tile.TileContext(nc) as tc:
        with (
            tc.tile_pool(name="const", bufs=1) as const,
            tc.tile_pool(name="wpool", bufs=1) as wpool,
        ):
            wqk = tc.alloc_tile_pool(name="wqk", bufs=1)
            # ============ PREP ============
            ident = const.tile([128, 128], BF16)
            masks.make_identity(nc, ident[:])
            zeros_bf = const.tile([128, 768], BF16)
            nc.vector.memset(zeros_bf[:], 0.0)

            # gamma rows and bias rows
            g1_row = const.tile([1, C], F32)
            nc.sync.dma_start(g1_row[:], gamma_1[None, :])
            g2_row = const.tile([1, C], F32)
            nc.sync.dma_start(g2_row[:], gamma_2[None, :])
            bproj_row = const.tile([1, C], F32)
            nc.sync.dma_start(bproj_row[:], b_proj[None, :])
            bfc2_row = const.tile([1, C], F32)
            nc.sync.dma_start(bfc2_row[:], b_fc2[None, :])

            # ln gains as [128, CC] feature-major (per-partition scalars)
            ln1g_fm = const.tile([128, CC], F32)
            nc.sync.dma_start(ln1g_fm[:], ln1_g.rearrange("(a p) -> p a", p=128))
            ln2g_fm = const.tile([128, CC], F32)
            nc.sync.dma_start(ln2g_fm[:], ln2_g.rearrange("(a p) -> p a", p=128))
            ln1b_fm = const.tile([128, CC], F32)
            nc.sync.dma_start(ln1b_fm[:], ln1_b.rearrange("(a p) -> p a", p=128))
            ln2b_fm = const.tile([128, CC], F32)
            nc.sync.dma_start(ln2b_fm[:], ln2_b.rearrange("(a p) -> p a", p=128))

            sincos = wqk.tile([128, 2 * TOK], BF16)  # [:, :TOK]=SIN, [:, TOK:]=COS
            SIN = sincos[:, 0:TOK]
            COS = sincos[:, TOK:2 * TOK]

            with (
                tc.tile_pool(name="prep_sb", bufs=1) as prep_sb,
                tc.tile_pool(name="prep_ps", bufs=1, space="PSUM") as prep_ps,
            ):
                # COS/SIN feature-major [128, 4096] bf16 (4 head-replicas of 32)
                cs_f = prep_sb.tile([64, TOK], F32, name="cs_f")
                nc.sync.dma_start(
                    cs_f[:],
                    rope_in.rearrange("r c d -> d (r c)"))
                for k in range(4):
                    nc.vector.tensor_copy(sincos[32 * k:32 * (k + 1), 0:TOK],
                                          cs_f[0:32, :])
                    nc.vector.tensor_copy(sincos[32 * k:32 * (k + 1), TOK:2 * TOK],
                                          cs_f[32:64, :])
                # gamma bcast tiles (for free-dim weight folds)
                g1b = prep_sb.tile([128, C], F32)
                nc.gpsimd.partition_broadcast(g1b[:], g1_row[:])
                g2b = prep_sb.tile([128, C], F32)
                nc.gpsimd.partition_broadcast(g2b[:], g2_row[:])

                # ---- weights: WqkT (permuted q,k + natural v), ln1_g fold ----
                wqkT = [wqk.tile([128, 3 * C], BF16, name=f"wqkT{c}")
                        for c in range(CC)]
                bias_ps = prep_ps.tile([128, 18], F32)
                lnb1_fm = [ln1b_fm[:, c:c + 1] for c in range(CC)]
                for j in range(18):
                    for c in range(CC):
                        raw = prep_sb.tile([128, 128], F32, tag="rawW", bufs=3,
                                         name=f"raw_{c}_{j}")
                        if j < 12:
                            sect, jj = divmod(j, 6)
                            half = jj // 3
                            h0 = 4 * (jj % 3)
                            base = sect * C + h0 * HD + half * 32
                            for hh in range(4):
                                r0_ = base + hh * HD
                                nc.sync.dma_start(
                                    raw[:, hh * 32:(hh + 1) * 32],
                                    w_qkv[r0_:r0_ + 32, c * 128:(c + 1) * 128]
                                    .rearrange("d c -> c d"))
                        else:
                            f0 = 2 * C + (j - 12) * 128
                            nc.sync.dma_start(
                                raw[:],
                                w_qkv[f0:f0 + 128, c * 128:(c + 1) * 128]
                                .rearrange("f c -> c f"))
                        # bias contribution: raw.T @ ln1_b  -> [128 f, 1]
                        nc.tensor.matmul(bias_ps[:, j:j + 1], raw[:], lnb1_fm[c],
                                         start=(c == 0), stop=(c == CC - 1))
                        # fold ln1_g (per-partition = per-C-row) and cast
                        nc.vector.tensor_scalar(
                            wqkT[c][:, j * 128:(j + 1) * 128], raw[:],
                            ln1g_fm[:, c:c + 1], None, op0=AluOpType.mult)
                bias_qkv_fm = wqk.tile([128, 18], F32)
                # b_qkv in permuted feature-major order
                bq_perm = wqk.tile([128, 18], F32)
                for j in range(12):
                    sect, jj = divmod(j, 6)
                    half = jj // 3
                    h0 = 4 * (jj % 3)
                    base = sect * C + h0 * HD + half * 32
                    for hh in range(4):
                        r0_ = base + hh * HD
                        nc.sync.dma_start(
                            bq_perm[hh * 32:(hh + 1) * 32, j:j + 1],
                            b_qkv[r0_:r0_ + 32, None])
                for j in range(12, 18):
                    f0 = 2 * C + (j - 12) * 128
                    nc.sync.dma_start(bq_perm[:, j:j + 1],
                                      b_qkv[f0:f0 + 128, None])
                nc.vector.tensor_tensor(bias_qkv_fm[:], bias_ps[:], bq_perm[:],
                                        op=AluOpType.add)

                # ---- W1T with ln2_g fold + bias ----
                w1T = [wpool.tile([128, HF], BF16, name=f"w1T{c}")
                       for c in range(CC)]
                bias1_ps = prep_ps.tile([128, 24], F32)
                for j in range(24):
                    for c in range(CC):
                        raw = prep_sb.tile([128, 128], F32, tag="rawW", bufs=3,
                                         name=f"raw1_{c}_{j}")
                        nc.sync.dma_start(
                            raw[:],
                            w_fc1[j * 128:(j + 1) * 128, c * 128:(c + 1) * 128]
                            .rearrange("f c -> c f"))
                        nc.tensor.matmul(bias1_ps[:, j:j + 1], raw[:],
                                         ln2b_fm[:, c:c + 1],
                                         start=(c == 0), stop=(c == CC - 1))
                        nc.vector.tensor_scalar(
                            w1T[c][:, j * 128:(j + 1) * 128], raw[:],
                            ln2g_fm[:, c:c + 1], None, op0=AluOpType.mult)
                bias_fc1_fm = const.tile([128, 24], F32)
                b1_fm = const.tile([128, 24], F32)
                nc.sync.dma_start(b1_fm[:], b_fc1.rearrange("(a p) -> p a", p=128))
                nc.vector.tensor_tensor(bias_fc1_fm[:], bias1_ps[:], b1_fm[:],
                                        op=AluOpType.add)

                # ---- WpT_eff (gamma_1 row fold) + bp_eff row ----
                wpT = [wpool.tile([128, C], BF16, name=f"wpT{c}")
                       for c in range(CC)]
                bv_fm = const.tile([128, CC], F32)
                nc.sync.dma_start(bv_fm[:],
                                  b_qkv[2 * C:3 * C].rearrange("(a p) -> p a", p=128))
                brow_ps = prep_ps.tile([1, C], F32)
                for c in range(CC):
                    rawp = prep_sb.tile([128, C], F32, tag="rawWp", bufs=2,
                                      name=f"rawp_{c}")
                    nc.sync.dma_start(rawp[:],
                                      w_proj[:, c * 128:(c + 1) * 128]
                                      .rearrange("f c -> c f"))
                    # Wp @ b_v row: lhsT = b_v_fm chunk [128,1], rhs = rawp
                    nc.tensor.matmul(brow_ps[:, 0:512], bv_fm[:, c:c + 1],
                                     rawp[:, 0:512],
                                     start=(c == 0), stop=(c == CC - 1))
                    nc.tensor.matmul(brow_ps[:, 512:768], bv_fm[:, c:c + 1],
                                     rawp[:, 512:768],
                                     start=(c == 0), stop=(c == CC - 1))
                    nc.vector.tensor_tensor(wpT[c][:], rawp[:], g1b[:],
                                            op=AluOpType.mult)
                bp_row = const.tile([1, C], F32)
                nc.vector.tensor_tensor(bp_row[:], brow_ps[:], bproj_row[:],
                                        op=AluOpType.add)
                nc.vector.tensor_tensor(bp_row[:], bp_row[:], g1_row[:],
                                        op=AluOpType.mult)
                bp_bcast = const.tile([128, C], F32)
                nc.gpsimd.partition_broadcast(bp_bcast[:], bp_row[:])

                # ---- W2T_eff (gamma_2 row fold) + b2 row ----
                w2T = [wpool.tile([128, C], BF16, name=f"w2T{j}")
                       for j in range(24)]
                for j in range(24):
                    raw2 = prep_sb.tile([128, C], F32, tag="rawWp", bufs=2,
                                      name=f"raw2_{j}")
                    nc.sync.dma_start(raw2[:],
                                      w_fc2[:, j * 128:(j + 1) * 128]
                                      .rearrange("f c -> c f"))
                    nc.vector.tensor_tensor(w2T[j][:], raw2[:], g2b[:],
                                            op=AluOpType.mult)
                b2_row = const.tile([1, C], F32)
                nc.vector.tensor_tensor(b2_row[:], bfc2_row[:], g2_row[:],
                                        op=AluOpType.mult)
                b2_bcast = const.tile([128, C], F32)
                nc.gpsimd.partition_broadcast(b2_bcast[:], b2_row[:])

            # ---- zero pad strips of q_dram/k_dram, pad tokens of v_dram ----
            for dram in (q_dram, k_dram):
                for h in range(NH):
                    nc.sync.dma_start(
                        dram[h, :, :, G:GP],
                        zeros_bf[0:HD, 0:GP * 6].rearrange("d (r c) -> d r c", c=6))
                    nc.sync.dma_start(
                        dram[h, :, G:GP, 0:G],
                        zeros_bf[0:HD, 0:6 * G].rearrange("d (r c) -> d r c", c=G))
            for wr in range(NW1):
                for wc in range(NW1):
                    w = wr * NW1 + wc
                    if wc == NW1 - 1:
                        # cols 8..13 of every row are padding
                        nc.sync.dma_start(
                            v_dram[w].rearrange("(r c) f -> r c f", c=WS)[:, 8:WS, :],
                            zeros_bf[0:84, :])
                    if wr == NW1 - 1:
                        # rows 8..13 (tokens 112:196) are padding
                        nc.sync.dma_start(v_dram[w, 112:T, :], zeros_bf[0:84, :])

            # ============ PHASE A ============
            with (
                tc.tile_pool(name="a_sb", bufs=2) as a_sb,
                tc.tile_pool(name="a_ps", bufs=2, space="PSUM") as a_ps,
                tc.tile_pool(name="a_tr_ps", bufs=2, space="PSUM") as a_tr_ps,
                tc.tile_pool(name="a_vps", bufs=2, space="PSUM") as a_vps,
            ):
                for ch in range(NCH):
                    xlnT = [a_sb.tile([128, CH], BF16, tag=f"xlnT{c}",
                                      name=f"xlnT_{ch}_{c}") for c in range(CC)]
                    for t4 in range(4):
                        tok0 = ch * CH + t4 * 128
                        xt = a_sb.tile([128, C], F32, tag="xt", bufs=2,
                                       name=f"xt_{ch}_{t4}")
                        nc.sync.dma_start(xt[:], x_t[tok0:tok0 + 128, :])
                        stats = a_sb.tile([128, 12], F32, tag="stats", bufs=3,
                                          name=f"st_{ch}_{t4}")
                        nc.vector.bn_stats(stats[:, 0:6], xt[:, 0:384])
                        nc.vector.bn_stats(stats[:, 6:12], xt[:, 384:768])
                        mv = a_sb.tile([128, 2], F32, tag="mv", bufs=3,
                                       name=f"mv_{ch}_{t4}")
                        nc.vector.bn_aggr(mv[:], stats[:])
                        rs = a_sb.tile([128, 1], F32, tag="rs", bufs=3,
                                       name=f"rs_{ch}_{t4}")
                        nc.vector.tensor_scalar_add(rs[:], mv[:, 1:2], LN_EPS)
                        nc.vector.reciprocal(rs[:], rs[:])
                        nc.scalar.sqrt(rs[:], rs[:])
                        xln = a_sb.tile([128, C], BF16, tag="xln", bufs=3,
                                        name=f"xln_{ch}_{t4}")
                        nc.vector.tensor_scalar(xln[:], xt[:], mv[:, 0:1], rs[:],
                                                op0=AluOpType.subtract,
                                                op1=AluOpType.mult)
                        for c in range(CC):
                            trp = a_tr_ps.tile([128, 128], BF16, tag="trp",
                                               name=f"trp_{ch}_{t4}_{c}")
                            nc.tensor.transpose(trp[:], xln[:, c * 128:(c + 1) * 128],
                                                ident[:])
                            nc.scalar.copy(xlnT[c][:, t4 * 128:(t4 + 1) * 128],
                                           trp[:])
                    # q,k feature-major with rope (pairwise to bound slots)
                    cosc = COS[:, ch * CH:(ch + 1) * CH]
                    sinc = SIN[:, ch * CH:(ch + 1) * CH]
                    for sect in range(2):
                        dram = q_dram if sect == 0 else k_dram
                        for jp in range(3):
                            pair = []
                            for j in (sect * 6 + jp, sect * 6 + jp + 3):
                                ps = a_ps.tile([128, CH], F32, tag="qkps",
                                               name=f"qkps_{ch}_{j}")
                                for c in range(CC):
                                    nc.tensor.matmul(
                                        ps[:], wqkT[c][:, j * 128:(j + 1) * 128],
                                        xlnT[c][:], start=(c == 0),
                                        stop=(c == CC - 1))
                                sb = a_sb.tile([128, CH], F32, tag="qksb", bufs=4,
                                               name=f"qksb_{ch}_{j}")
                                nc.scalar.activation(sb[:], ps[:], AF.Identity,
                                                     bias=bias_qkv_fm[:, j:j + 1])
                                pair.append(sb)
                            q1, q2 = pair
                            t1 = a_sb.tile([128, CH], F32, tag="ropet1", bufs=2,
                                           name=f"t1_{ch}_{sect}_{jp}")
                            t2 = a_sb.tile([128, CH], F32, tag="ropet2", bufs=2,
                                           name=f"t2_{ch}_{sect}_{jp}")
                            r1 = a_sb.tile([128, CH], BF16, tag="roper1", bufs=2,
                                           name=f"r1_{ch}_{sect}_{jp}")
                            r2 = a_sb.tile([128, CH], BF16, tag="roper2", bufs=2,
                                           name=f"r2_{ch}_{sect}_{jp}")
                            nc.vector.tensor_tensor(t1[:], q1[:], cosc,
                                                    op=AluOpType.mult)
                            nc.vector.tensor_tensor(t2[:], q2[:], sinc,
                                                    op=AluOpType.mult)
                            nc.vector.tensor_tensor(r1[:], t1[:], t2[:],
                                                    op=AluOpType.subtract)
                            nc.vector.tensor_tensor(t1[:], q2[:], cosc,
                                                    op=AluOpType.mult)
                            nc.vector.tensor_tensor(t2[:], q1[:], sinc,
                                                    op=AluOpType.mult)
                            nc.vector.tensor_tensor(r2[:], t1[:], t2[:],
                                                    op=AluOpType.add)
                            # store: rows of r1 are heads 4jp..4jp+3, d 0..31
                            r0 = ch * 8  # 8 grid rows per chunk
                            eng = nc.sync if sect == 0 else nc.scalar
                            for hh in range(4):
                                h = 4 * jp + hh
                                eng.dma_start(
                                    dram[h, 0:32, r0:r0 + 8, 0:G],
                                    r1[32 * hh:32 * (hh + 1), :]
                                    .rearrange("d (r c) -> d r c", c=G))
                                eng.dma_start(
                                    dram[h, 32:64, r0:r0 + 8, 0:G],
                                    r2[32 * hh:32 * (hh + 1), :]
                                    .rearrange("d (r c) -> d r c", c=G))
                    # v token-major
                    for t4 in range(4):
                        tok0 = ch * CH + t4 * 128
                        vbf = a_sb.tile([128, C], BF16, tag="vbf", bufs=2,
                                        name=f"vbf_{ch}_{t4}")
                        for nchunk, (f0, fn) in enumerate(((0, 512), (512, 256))):
                            vps = a_vps.tile([128, 512], F32, tag="vps",
                                             name=f"vps_{ch}_{t4}_{nchunk}")
                            for c in range(CC):
                                nc.tensor.matmul(
                                    vps[:, 0:fn],
                                    xlnT[c][:, t4 * 128:(t4 + 1) * 128],
                                    wqkT[c][:, 2 * C + f0:2 * C + f0 + fn],
                                    start=(c == 0), stop=(c == CC - 1))
                            nc.scalar.copy(vbf[:, f0:f0 + fn], vps[:, 0:fn])
                        # scatter rows into window scratch
                        gr0 = (ch * CH + t4 * 128) // G  # first grid row
                        for rr in range(2):
                            r = gr0 + rr
                            wr, r_in = divmod(r, WS)
                            for wc in range(NW1):
                                w = wr * NW1 + wc
                                cw = 8 if wc == NW1 - 1 else WS
                                nc.gpsimd.dma_start(
                                    v_dram[w, r_in * WS:r_in * WS + cw, :],
                                    vbf[rr * 64 + wc * WS:rr * 64 + wc * WS + cw, :])

            wqk.release()

            # ============ PHASE B: attention ============
            with (
                tc.tile_pool(name="b_sb", bufs=4) as b_sb,
                tc.tile_pool(name="b_ps", bufs=4, space="PSUM") as b_ps,
                tc.tile_pool(name="b_ops", bufs=4, space="PSUM") as b_ops,
            ):
                for w in range(NW):
                    wr, wc = divmod(w, NW1)
                    r0, c0 = wr * WS, wc * WS
                    for h in range(NH):
                        qw = b_sb.tile([HD, T], BF16, tag="qw",
                                       name=f"qw_{w}_{h}")
                        kw = b_sb.tile([HD, T], BF16, tag="kw",
                                       name=f"kw_{w}_{h}")
                        nc.sync.dma_start(
                            qw[:].rearrange("d (r c) -> d r c", c=WS),
                            q_dram[h, :, r0:r0 + WS, c0:c0 + WS])
                        nc.scalar.dma_start(
                            kw[:].rearrange("d (r c) -> d r c", c=WS),
                            k_dram[h, :, r0:r0 + WS, c0:c0 + WS])
                        v0 = b_sb.tile([KC0, 65], BF16, tag="v0",
                                       name=f"v0_{w}_{h}")
                        v1 = b_sb.tile([KC1, 65], BF16, tag="v1",
                                       name=f"v1_{w}_{h}")
                        nc.scalar.dma_start(v0[:, 0:64],
                                          v_dram[w, 0:KC0, h * HD:(h + 1) * HD])
                        nc.scalar.dma_start(v1[:, 0:64],
                                          v_dram[w, KC0:T, h * HD:(h + 1) * HD])
                        nc.vector.memset(v0[:, 64:65], 1.0)
                        nc.vector.memset(v1[:, 64:65], 1.0)
                        s0 = b_ps.tile([KC0, T], F32, tag="s", name=f"s0_{w}_{h}")
                        s1 = b_ps.tile([KC1, T], F32, tag="s", name=f"s1_{w}_{h}")
                        nc.tensor.matmul(s0[:], kw[:, 0:KC0], qw[:],
                                         start=True, stop=True)
                        nc.tensor.matmul(s1[:], kw[:, KC0:T], qw[:],
                                         start=True, stop=True)
                        e0 = b_sb.tile([KC0, T], BF16, tag="e0",
                                       name=f"e0_{w}_{h}")
                        e1 = b_sb.tile([KC1, T], BF16, tag="e1",
                                       name=f"e1_{w}_{h}")
                        nc.scalar.activation(e0[:], s0[:], AF.Exp, scale=HD ** -0.5)
                        nc.scalar.activation(e1[:], s1[:], AF.Exp, scale=HD ** -0.5)
                        o = b_ops.tile([65, T], F32, tag="o", name=f"o_{w}_{h}")
                        nc.tensor.matmul(o[:], v0[:], e0[:], start=True, stop=False)
                        nc.tensor.matmul(o[:], v1[:], e1[:], start=False, stop=True)
                        rcp = b_sb.tile([1, T], F32, tag="rcp", name=f"rcp_{w}_{h}")
                        nc.vector.reciprocal(rcp[:], o[64:65, :])
                        rb = b_sb.tile([HD, T], F32, tag="rb", name=f"rb_{w}_{h}")
                        nc.gpsimd.partition_broadcast(rb[:], rcp[:])
                        ab = b_sb.tile([HD, T], BF16, tag="ab", name=f"ab_{w}_{h}")
                        nc.vector.tensor_tensor(ab[:], o[0:64, :], rb[:],
                                                op=AluOpType.mult)
                        nc.sync.dma_start(
                            att_dram[h * HD:(h + 1) * HD, r0:r0 + WS, c0:c0 + WS],
                            ab[:].rearrange("d (r c) -> d r c", c=WS))

            # ============ PHASE C: proj + residual + LN2 + transpose ============
            with (
                tc.tile_pool(name="c_sb", bufs=2) as c_sb,
                tc.tile_pool(name="c_ps", bufs=2, space="PSUM") as c_ps,
                tc.tile_pool(name="c_tr_ps", bufs=2, space="PSUM") as c_tr_ps,
            ):
                for ch in range(NCH):
                    yT = [c_sb.tile([128, CH], BF16, tag=f"yT{c}",
                                    name=f"yT_{ch}_{c}") for c in range(CC)]
                    for t4 in range(4):
                        tok0 = ch * CH + t4 * 128
                        gr0 = tok0 // G
                        attT = []
                        for c in range(CC):
                            at = c_sb.tile([128, 128], BF16, tag=f"attT{c}", bufs=2,
                                           name=f"attT_{ch}_{t4}_{c}")
                            nc.sync.dma_start(
                                at[:].rearrange("f (r c) -> f r c", c=G),
                                att_dram[c * 128:(c + 1) * 128, gr0:gr0 + 2, 0:G])
                            attT.append(at)
                        x1 = c_sb.tile([128, C], F32, tag="x1", bufs=2,
                                       name=f"x1_{ch}_{t4}")
                        xt = c_sb.tile([128, C], F32, tag="xt2", bufs=2,
                                       name=f"xt2_{ch}_{t4}")
                        nc.sync.dma_start(xt[:], x_t[tok0:tok0 + 128, :])
                        for f0, fn in ((0, 512), (512, 256)):
                            pps = c_ps.tile([128, 512], F32, tag="pps",
                                            name=f"pps_{ch}_{t4}_{f0}")
                            for c in range(CC):
                                nc.tensor.matmul(pps[:, 0:fn], attT[c][:],
                                                 wpT[c][:, f0:f0 + fn],
                                                 start=(c == 0), stop=(c == CC - 1))
                            nc.vector.scalar_tensor_tensor(
                                x1[:, f0:f0 + fn], pps[:, 0:fn], 1.0,
                                bp_bcast[:, f0:f0 + fn],
                                op0=AluOpType.mult, op1=AluOpType.add)
                        nc.vector.tensor_tensor(x1[:], x1[:], xt[:],
                                                op=AluOpType.add)
                        nc.sync.dma_start(x1_dram[tok0:tok0 + 128, :], x1[:])
                        # LN2
                        stats = c_sb.tile([128, 12], F32, tag="stats2", bufs=3,
                                          name=f"st2_{ch}_{t4}")
                        nc.vector.bn_stats(stats[:, 0:6], x1[:, 0:384])
                        nc.vector.bn_stats(stats[:, 6:12], x1[:, 384:768])
                        mv = c_sb.tile([128, 2], F32, tag="mv2", bufs=3,
                                       name=f"mv2_{ch}_{t4}")
                        nc.vector.bn_aggr(mv[:], stats[:])
                        rs = c_sb.tile([128, 1], F32, tag="rs2", bufs=3,
                                       name=f"rs2_{ch}_{t4}")
                        nc.vector.tensor_scalar_add(rs[:], mv[:, 1:2], LN_EPS)
                        nc.vector.reciprocal(rs[:], rs[:])
                        nc.scalar.sqrt(rs[:], rs[:])
                        y = c_sb.tile([128, C], BF16, tag="y", bufs=3,
                                      name=f"y_{ch}_{t4}")
                        nc.vector.tensor_scalar(y[:], x1[:], mv[:, 0:1], rs[:],
                                                op0=AluOpType.subtract,
                                                op1=AluOpType.mult)
                        for c in range(CC):
                            trp = c_tr_ps.tile([128, 128], BF16, tag="trp2",
                                               name=f"trp2_{ch}_{t4}_{c}")
                            nc.tensor.transpose(trp[:], y[:, c * 128:(c + 1) * 128],
                                                ident[:])
                            nc.scalar.copy(yT[c][:, t4 * 128:(t4 + 1) * 128],
                                           trp[:])
                    # ============ PHASE D (per chunk): MLP ============
                    h1 = [c_sb.tile([128, CH], BF16, tag=f"h1_{j}", bufs=1,
                                    name=f"h1_{ch}_{j}") for j in range(24)]
                    for j in range(24):
                        hps = c_ps.tile([128, 512], F32, tag="hps",
                                        name=f"hps_{ch}_{j}")
                        for c in range(CC):
                            nc.tensor.matmul(hps[:], w1T[c][:, j * 128:(j + 1) * 128],
                                             yT[c][:], start=(c == 0),
                                             stop=(c == CC - 1))
                        if not sim_gelu:
                            nc.scalar.activation(h1[j][:], hps[:], AF.Gelu,
                                                 bias=bias_fc1_fm[:, j:j + 1])
                        else:
                            # CoreSim lacks Gelu: tanh-approx decomposition
                            tg = c_sb.tile([128, CH], F32, tag="tg", bufs=2,
                                           name=f"tg_{ch}_{j}")
                            nc.scalar.activation(tg[:], hps[:], AF.Identity,
                                                 bias=bias_fc1_fm[:, j:j + 1])
                            sq = c_sb.tile([128, CH], F32, tag="sq", bufs=2,
                                           name=f"sq_{ch}_{j}")
                            nc.scalar.activation(sq[:], tg[:], AF.Square)
                            nc.vector.tensor_scalar(sq[:], sq[:], 0.044715, 1.0,
                                                    op0=AluOpType.mult,
                                                    op1=AluOpType.add)
                            nc.vector.tensor_tensor(sq[:], sq[:], tg[:],
                                                    op=AluOpType.mult)
                            nc.scalar.activation(sq[:], sq[:], AF.Tanh,
                                                 scale=0.7978845608028654)
                            nc.vector.tensor_scalar(sq[:], sq[:], 1.0, 0.5,
                                                    op0=AluOpType.add,
                                                    op1=AluOpType.mult)
                            nc.vector.tensor_tensor(h1[j][:], sq[:], tg[:],
                                                    op=AluOpType.mult)
                    for t4 in range(4):
                        tok0 = ch * CH + t4 * 128
                        x1t = c_sb.tile([128, C], F32, tag="x1t", bufs=2,
                                        name=f"x1t_{ch}_{t4}")
                        nc.sync.dma_start(x1t[:], x1_dram[tok0:tok0 + 128, :])
                        ot = c_sb.tile([128, C], F32, tag="ot", bufs=2,
                                       name=f"ot_{ch}_{t4}")
                        for f0, fn in ((0, 512), (512, 256)):
                            ops_ = c_ps.tile([128, 512], F32, tag="ops",
                                             name=f"ops_{ch}_{t4}_{f0}")
                            for j in range(24):
                                nc.tensor.matmul(
                                    ops_[:, 0:fn],
                                    h1[j][:, t4 * 128:(t4 + 1) * 128],
                                    w2T[j][:, f0:f0 + fn],
                                    start=(j == 0), stop=(j == 23))
                            nc.vector.scalar_tensor_tensor(
                                ot[:, f0:f0 + fn], ops_[:, 0:fn], 1.0,
                                b2_bcast[:, f0:f0 + fn],
                                op0=AluOpType.mult, op1=AluOpType.add)
                        nc.vector.tensor_tensor(ot[:], ot[:], x1t[:],
                                                op=AluOpType.add)
                        nc.sync.dma_start(out_t[tok0:tok0 + 128, :], ot[:])

    nc.finalize()
    return nc


def kernel(**inputs) -> np.ndarray:
    global _COMPILED
    from concourse.bass_utils import run_bass_kernel_spmd

    if _COMPILED is None:
        _COMPILED = _build()
    nc = _COMPILED

    x = np.ascontiguousarray(np.asarray(inputs["x"], dtype=np.float32))
    rope = np.ascontiguousarray(
        np.asarray(inputs["rope_2d"], dtype=np.float32).reshape(G, G, HD))
    shared = {
        k: np.ascontiguousarray(np.asarray(inputs[k], dtype=np.float32))
        for k in ("ln1_g", "ln1_b", "w_qkv", "b_qkv", "w_proj", "b_proj",
                  "gamma_1", "ln2_g", "ln2_b", "w_fc1", "b_fc1", "w_fc2",
                  "b_fc2", "gamma_2")
    }
    in_maps = [{"x": x[b], "rope_2d": rope, **shared} for b in range(8)]
    res = run_bass_kernel_spmd(nc, in_maps, list(range(8)))
    return np.stack([res.results[b]["out"] for b in range(8)]).astype(np.float32)


# revision 31
# speedup vs baseline: 4701.3884x; 4691.9017x over previous
"""Trainium2 Bass kernel for nn_Block_47880295416554 (windowed-attention
transformer block with RoPE, EVA/Swin style).

Sharding: data-parallel over batch B=8 across the 8 NeuronCores; weights
replicated. Each core runs the full block on one [64, 64, 768] image.

Per-core pipeline (all matmuls in bf16, fp32 accumulate; residual spine fp32):
  A: LN1 (g/b folded into qkv weights/bias) -> PE-transpose -> q,k feature-
     major + RoPE -> padded-grid scratch; v token-major -> window scratch.
  B: per (window, head): scoresT = k_win^T q_win on PE, exp on ACT (no max
     subtraction; scores are small), out = v_aug^T @ expT with a ones column
     giving the softmax denominator; normalize; store feature-major.
  C: proj (gamma_1 folded into weights, b_v folded into bias) + residual,
     LN2, transpose.
  D: MLP fc1+gelu (feature-major) then fc2 (gamma_2 folded) + residual.
"""
import numpy as np

C = 768
G = 64          # grid H = W
GP = 70         # padded grid (5 windows of 14)
WS = 14
NW1 = 5
NW = 25
T = 196         # tokens per window
NH = 12
HD = 64
HF = 3072
TOK = 4096
CH = 512        # token chunk
NCH = 8
CC = 6          # C / 128
LN_EPS = 1e-6
KC0, KC1 = 126, 70   # window token chunks (9 rows, 5 rows)

_COMPILED = None


def _build(sim_gelu=False, trace_sim=False):
    import concourse.bacc as bacc
    import concourse.mybir as mybir
    from concourse import tile, masks
    from concourse.alu_op_type import AluOpType

    F32 = mybir.dt.float32
    BF16 = mybir.dt.bfloat16
    AF = mybir.ActivationFunctionType

    nc = bacc.Bacc(None, target_bir_lowering=False, debug=False)

    # ---- I/O ----
    x_in = nc.declare_dram_parameter("x", [G, G, C], F32, isOutput=False)
    rope_in = nc.declare_dram_parameter("rope_2d", [G, G, HD], F32, isOutput=False)
    ln1_g = nc.declare_dram_parameter("ln1_g", [C], F32, isOutput=False)
    ln1_b = nc.declare_dram_parameter("ln1_b", [C], F32, isOutput=False)
    w_qkv = nc.declare_dram_parameter("w_qkv", [3 * C, C], F32, isOutput=False)
    b_qkv = nc.declare_dram_parameter("b_qkv", [3 * C], F32, isOutput=False)
    w_proj = nc.declare_dram_parameter("w_proj", [C, C], F32, isOutput=False)
    b_proj = nc.declare_dram_parameter("b_proj", [C], F32, isOutput=False)
    gamma_1 = nc.declare_dram_parameter("gamma_1", [C], F32, isOutput=False)
    ln2_g = nc.declare_dram_parameter("ln2_g", [C], F32, isOutput=False)
    ln2_b = nc.declare_dram_parameter("ln2_b", [C], F32, isOutput=False)
    w_fc1 = nc.declare_dram_parameter("w_fc1", [HF, C], F32, isOutput=False)
    b_fc1 = nc.declare_dram_parameter("b_fc1", [HF], F32, isOutput=False)
    w_fc2 = nc.declare_dram_parameter("w_fc2", [C, HF], F32, isOutput=False)
    b_fc2 = nc.declare_dram_parameter("b_fc2", [C], F32, isOutput=False)
    gamma_2 = nc.declare_dram_parameter("gamma_2", [C], F32, isOutput=False)
    out = nc.declare_dram_parameter("out", [G, G, C], F32, isOutput=True)

    # ---- DRAM scratch ----
    q_dram = nc.dram_tensor("q_dram", [NH // 2, 2 * HD, GP, GP], BF16)
    k_dram = nc.dram_tensor("k_dram", [NH // 2, 2 * HD, GP, GP], BF16)
    v_dram = nc.dram_tensor("v_dram", [NW, T, C], BF16)
    att_dram = nc.dram_tensor("att_dram", [CC, 128, GP, GP], BF16)
    x1_dram = nc.dram_tensor("x1_dram", [TOK, C], F32)

    x_t = x_in.rearrange("r c d -> (r c) d")      # [4096, 768]
    out_t = out.rearrange("r c d -> (r c) d")


    with tile.TileContext(nc, trace_sim=trace_sim) as tc:
        with (
            tc.tile_pool(name="const", bufs=1) as const,
            tc.tile_pool(name="wpool", bufs=1) as wpool,
            tc.tile_pool(name="ps", bufs=1, space="PSUM") as ps_pool,
            tc.tile_pool(name="b_sb", bufs=3) as b_sb,
        ):
            wqk = tc.alloc_tile_pool(name="wqk", bufs=1)
            # ============ PREP ============
            ident = const.tile([128, 128], BF16)
            masks.make_identity(nc, ident[:])
            zeros_bf = const.tile([128, 768], BF16)
            nc.vector.memset(zeros_bf[:], 0.0)
            ones_col = const.tile([128, 1], BF16)
            nc.vector.memset(ones_col[:], 1.0)

            # gamma rows and bias rows
            g1_row = const.tile([1, C], F32)
            nc.sync.dma_start(g1_row[:], gamma_1[None, :])
            g2_row = const.tile([1, C], F32)
            nc.sync.dma_start(g2_row[:], gamma_2[None, :])
            bproj_row = const.tile([1, C], F32)
            nc.sync.dma_start(bproj_row[:], b_proj[None, :])
            bfc2_row = const.tile([1, C], F32)
            nc.sync.dma_start(bfc2_row[:], b_fc2[None, :])

            # ln gains as [128, CC] feature-major (per-partition scalars)
            ln1g_fm = const.tile([128, CC], F32)
            nc.sync.dma_start(ln1g_fm[:], ln1_g.rearrange("(a p) -> p a", p=128))
            ln2g_fm = const.tile([128, CC], F32)
            nc.sync.dma_start(ln2g_fm[:], ln2_g.rearrange("(a p) -> p a", p=128))
            ln1b_fm = const.tile([128, CC], F32)
            nc.sync.dma_start(ln1b_fm[:], ln1_b.rearrange("(a p) -> p a", p=128))
            ln2b_fm = const.tile([128, CC], F32)
            nc.sync.dma_start(ln2b_fm[:], ln2_b.rearrange("(a p) -> p a", p=128))

            sincos = wqk.tile([128, 2 * TOK], BF16)  # [:, :TOK]=SIN, [:, TOK:]=COS
            SIN = sincos[:, 0:TOK]
            COS = sincos[:, TOK:2 * TOK]

            with tc.tile_pool(name="prep_sb", bufs=1) as prep_sb:
                prep_ps = ps_pool
                # COS/SIN feature-major [128, 4096] bf16 (4 head-replicas of 32)
                cs_f = prep_sb.tile([64, TOK], F32, name="cs_f")
                nc.sync.dma_start(
                    cs_f[:],
                    rope_in.rearrange("r c d -> d (r c)"))
                for k in range(4):
                    nc.vector.tensor_copy(sincos[32 * k:32 * (k + 1), 0:TOK],
                                          cs_f[0:32, :])
                    nc.vector.tensor_copy(sincos[32 * k:32 * (k + 1), TOK:2 * TOK],
                                          cs_f[32:64, :])
                # gamma bcast tiles (for free-dim weight folds)
                g1b = prep_sb.tile([128, C], F32)
                nc.gpsimd.partition_broadcast(g1b[:], g1_row[:])
                g2b = prep_sb.tile([128, C], F32)
                nc.gpsimd.partition_broadcast(g2b[:], g2_row[:])

                # ---- weights: WqkT (permuted q,k + natural v), ln1_g fold ----
                wqkT = [wqk.tile([128, 3 * C], BF16, name=f"wqkT{c}")
                        for c in range(CC)]
                bias_ps = prep_ps.tile([128, 18], F32, tag="tr", bufs=1)
                lnb1_fm = [ln1b_fm[:, c:c + 1] for c in range(CC)]
                for j in range(18):
                    for c in range(CC):
                        raw = prep_sb.tile([128, 128], F32, tag="rawW", bufs=3,
                                         name=f"raw_{c}_{j}")
                        if j < 12:
                            sect, jj = divmod(j, 6)
                            half = jj // 3
                            h0 = 4 * (jj % 3)
                            base = sect * C + h0 * HD + half * 32
                            weng = nc.gpsimd if sect == 0 else nc.sync
                            for hh in range(4):
                                r0_ = base + hh * HD
                                weng.dma_start(
                                    raw[:, hh * 32:(hh + 1) * 32],
                                    w_qkv[r0_:r0_ + 32, c * 128:(c + 1) * 128]
                                    .rearrange("d c -> c d"))
                        else:
                            f0 = 2 * C + (j - 12) * 128
                            nc.scalar.dma_start(
                                raw[:],
                                w_qkv[f0:f0 + 128, c * 128:(c + 1) * 128]
                                .rearrange("f c -> c f"))
                        # bias contribution: raw.T @ ln1_b  -> [128 f, 1]
                        nc.tensor.matmul(bias_ps[:, j:j + 1], raw[:], lnb1_fm[c],
                                         start=(c == 0), stop=(c == CC - 1))
                        # fold ln1_g (per-partition = per-C-row) and cast
                        nc.vector.tensor_scalar(
                            wqkT[c][:, j * 128:(j + 1) * 128], raw[:],
                            ln1g_fm[:, c:c + 1], None, op0=AluOpType.mult)
                bias_qkv_fm = wqk.tile([128, 18], F32)
                # b_qkv in permuted feature-major order
                bq_perm = wqk.tile([128, 18], F32)
                for j in range(12):
                    sect, jj = divmod(j, 6)
                    half = jj // 3
                    h0 = 4 * (jj % 3)
                    base = sect * C + h0 * HD + half * 32
                    for hh in range(4):
                        r0_ = base + hh * HD
                        nc.gpsimd.dma_start(
                            bq_perm[hh * 32:(hh + 1) * 32, j:j + 1],
                            b_qkv[r0_:r0_ + 32, None])
                for j in range(12, 18):
                    f0 = 2 * C + (j - 12) * 128
                    nc.sync.dma_start(bq_perm[:, j:j + 1],
                                      b_qkv[f0:f0 + 128, None])
                nc.vector.tensor_tensor(bias_qkv_fm[:], bias_ps[:], bq_perm[:],
                                        op=AluOpType.add)

                # ---- W1T with ln2_g fold + bias ----
                w1T = [wpool.tile([128, HF], BF16, name=f"w1T{c}")
                       for c in range(CC)]
                bias1_ps = prep_ps.tile([128, 24], F32, tag="tr", bufs=1)
                for j in range(24):
                    for c in range(CC):
                        raw = prep_sb.tile([128, 128], F32, tag="rawW", bufs=3,
                                         name=f"raw1_{c}_{j}")
                        nc.scalar.dma_start(
                            raw[:],
                            w_fc1[j * 128:(j + 1) * 128, c * 128:(c + 1) * 128]
                            .rearrange("f c -> c f"))
                        nc.tensor.matmul(bias1_ps[:, j:j + 1], raw[:],
                                         ln2b_fm[:, c:c + 1],
                                         start=(c == 0), stop=(c == CC - 1))
                        nc.vector.tensor_scalar(
                            w1T[c][:, j * 128:(j + 1) * 128], raw[:],
                            ln2g_fm[:, c:c + 1], None, op0=AluOpType.mult)
                bias_fc1_fm = const.tile([128, 24], F32)
                b1_fm = const.tile([128, 24], F32)
                nc.sync.dma_start(b1_fm[:], b_fc1.rearrange("(a p) -> p a", p=128))
                nc.vector.tensor_tensor(bias_fc1_fm[:], bias1_ps[:], b1_fm[:],
                                        op=AluOpType.add)

                # ---- WpT_eff (gamma_1 row fold) + bp_eff row ----
                wpT = [wpool.tile([128, C], BF16, name=f"wpT{c}")
                       for c in range(CC)]
                bv_fm = const.tile([128, CC], F32)
                nc.sync.dma_start(bv_fm[:],
                                  b_qkv[2 * C:3 * C].rearrange("(a p) -> p a", p=128))
                brow_ps = prep_ps.tile([1, 512], F32, tag="mm512", bufs=3,
                                       name="brow_ps0")
                brow_ps1 = prep_ps.tile([1, 256], F32, tag="mm512", bufs=3,
                                        name="brow_ps1")
                for c in range(CC):
                    rawp = prep_sb.tile([128, C], F32, tag="rawWp", bufs=2,
                                      name=f"rawp_{c}")
                    nc.scalar.dma_start(rawp[:],
                                      w_proj[:, c * 128:(c + 1) * 128]
                                      .rearrange("f c -> c f"))
                    # Wp @ b_v row: lhsT = b_v_fm chunk [128,1], rhs = rawp
                    nc.tensor.matmul(brow_ps[:, 0:512], bv_fm[:, c:c + 1],
                                     rawp[:, 0:512],
                                     start=(c == 0), stop=(c == CC - 1))
                    nc.tensor.matmul(brow_ps1[:], bv_fm[:, c:c + 1],
                                     rawp[:, 512:768],
                                     start=(c == 0), stop=(c == CC - 1))
                    nc.vector.tensor_tensor(wpT[c][:], rawp[:], g1b[:],
                                            op=AluOpType.mult)
                bp_row = const.tile([1, C], F32)
                nc.vector.tensor_tensor(bp_row[:, 0:512], brow_ps[:],
                                        bproj_row[:, 0:512], op=AluOpType.add)
                nc.vector.tensor_tensor(bp_row[:, 512:768], brow_ps1[:],
                                        bproj_row[:, 512:768], op=AluOpType.add)
                nc.vector.tensor_tensor(bp_row[:], bp_row[:], g1_row[:],
                                        op=AluOpType.mult)
                bp_bcast = const.tile([128, C], F32)
                nc.gpsimd.partition_broadcast(bp_bcast[:], bp_row[:])

                # ---- W2T_eff (gamma_2 row fold) + b2 row ----
                w2T = [wpool.tile([128, C], BF16, name=f"w2T{j}")
                       for j in range(24)]
                for j in range(24):
                    raw2 = prep_sb.tile([128, C], F32, tag="rawWp", bufs=2,
                                      name=f"raw2_{j}")
                    nc.scalar.dma_start(raw2[:],
                                      w_fc2[:, j * 128:(j + 1) * 128]
                                      .rearrange("f c -> c f"))
                    nc.vector.tensor_tensor(w2T[j][:], raw2[:], g2b[:],
                                            op=AluOpType.mult)
                b2_row = const.tile([1, C], F32)
                nc.vector.tensor_tensor(b2_row[:], bfc2_row[:], g2_row[:],
                                        op=AluOpType.mult)
                b2_bcast = const.tile([128, C], F32)
                nc.gpsimd.partition_broadcast(b2_bcast[:], b2_row[:])

            # ---- zero pad strips of q_dram/k_dram, pad tokens of v_dram ----
            for dram in (q_dram, k_dram):
                for hp in range(NH // 2):
                    nc.scalar.dma_start(
                        dram[hp, :, :, G:GP],
                        zeros_bf[0:128, 0:GP * 6].rearrange("d (r c) -> d r c", c=6))
                    nc.scalar.dma_start(
                        dram[hp, :, G:GP, 0:G],
                        zeros_bf[0:128, 0:6 * G].rearrange("d (r c) -> d r c", c=G))
            for wr in range(NW1):
                for wc in range(NW1):
                    w = wr * NW1 + wc
                    if wc == NW1 - 1:
                        # cols 8..13 of every row are padding
                        nc.scalar.dma_start(
                            v_dram[w].rearrange("(r c) f -> r c f", c=WS)[:, 8:WS, :],
                            zeros_bf[0:84, :])
                    if wr == NW1 - 1:
                        # rows 8..13 (tokens 112:196) are padding
                        nc.scalar.dma_start(v_dram[w, 112:T, :], zeros_bf[0:84, :])

            # ============ MAIN (interleaved A/B/C/D) ============
            if True:
                a_sb = tc.alloc_tile_pool(name="a_sb", bufs=2)
                _pools = {}
                a_ps = ps_pool
                a_tr_ps = ps_pool
                a_vps = ps_pool
                b_ps = ps_pool
                b_ops = ps_pool
                c_ps = ps_pool
                c_tr_ps = ps_pool

                def do_A(ch):
                    xlnT = [a_sb.tile([128, CH], BF16, tag=f"xlnT{c}",
                                      name=f"xlnT_{ch}_{c}") for c in range(CC)]
                    for t4 in range(4):
                        tok0 = ch * CH + t4 * 128
                        xt = a_sb.tile([128, C], F32, tag="xt", bufs=2,
                                       name=f"xt_{ch}_{t4}")
                        nc.sync.dma_start(xt[:], x_t[tok0:tok0 + 128, :])
                        stats = a_sb.tile([128, 12], F32, tag="stats", bufs=3,
                                          name=f"st_{ch}_{t4}")
                        nc.vector.bn_stats(stats[:, 0:6], xt[:, 0:384])
                        nc.vector.bn_stats(stats[:, 6:12], xt[:, 384:768])
                        mv = a_sb.tile([128, 2], F32, tag="mv", bufs=3,
                                       name=f"mv_{ch}_{t4}")
                        nc.vector.bn_aggr(mv[:], stats[:])
                        rs = a_sb.tile([128, 1], F32, tag="rs", bufs=3,
                                       name=f"rs_{ch}_{t4}")
                        nc.vector.tensor_scalar_add(rs[:], mv[:, 1:2], LN_EPS)
                        nc.vector.reciprocal(rs[:], rs[:])
                        nc.scalar.sqrt(rs[:], rs[:])
                        xln = a_sb.tile([128, C], BF16, tag="xln", bufs=3,
                                        name=f"xln_{ch}_{t4}")
                        nc.vector.tensor_scalar(xln[:], xt[:], mv[:, 0:1], rs[:],
                                                op0=AluOpType.subtract,
                                                op1=AluOpType.mult)
                        for c in range(CC):
                            trp = a_tr_ps.tile([128, 128], BF16, tag="tr", bufs=1,
                                               name=f"trp_{ch}_{t4}_{c}")
                            nc.tensor.transpose(trp[:], xln[:, c * 128:(c + 1) * 128],
                                                ident[:])
                            nc.vector.tensor_copy(
                                xlnT[c][:, t4 * 128:(t4 + 1) * 128], trp[:])
                    # q,k feature-major with rope (pairwise to bound slots)
                    cosc = COS[:, ch * CH:(ch + 1) * CH]
                    sinc = SIN[:, ch * CH:(ch + 1) * CH]
                    for sect in range(2):
                        dram = q_dram if sect == 0 else k_dram
                        for jp in range(3):
                            pair = []
                            for j in (sect * 6 + jp, sect * 6 + jp + 3):
                                ps = a_ps.tile([128, CH], F32, tag="mm512", bufs=3,
                                               name=f"qkps_{ch}_{j}")
                                for c in range(CC):
                                    nc.tensor.matmul(
                                        ps[:], wqkT[c][:, j * 128:(j + 1) * 128],
                                        xlnT[c][:], start=(c == 0),
                                        stop=(c == CC - 1))
                                sb = a_sb.tile([128, CH], F32, tag="qksb", bufs=3,
                                               name=f"qksb_{ch}_{j}")
                                nc.vector.tensor_scalar(
                                    sb[:], ps[:], bias_qkv_fm[:, j:j + 1], None,
                                    op0=AluOpType.add)
                                pair.append(sb)
                            q1, q2 = pair
                            t1 = a_sb.tile([128, CH], F32, tag="ropet1", bufs=2,
                                           name=f"t1_{ch}_{sect}_{jp}")
                            t2 = a_sb.tile([128, CH], F32, tag="ropet2", bufs=2,
                                           name=f"t2_{ch}_{sect}_{jp}")
                            r1 = a_sb.tile([128, CH], BF16, tag="roper1", bufs=2,
                                           name=f"r1_{ch}_{sect}_{jp}")
                            r2 = a_sb.tile([128, CH], BF16, tag="roper2", bufs=2,
                                           name=f"r2_{ch}_{sect}_{jp}")
                            nc.vector.tensor_tensor(t1[:], q1[:], cosc,
                                                    op=AluOpType.mult)
                            nc.vector.tensor_tensor(t2[:], q2[:], sinc,
                                                    op=AluOpType.mult)
                            nc.vector.tensor_tensor(r1[:], t1[:], t2[:],
                                                    op=AluOpType.subtract)
                            nc.vector.tensor_tensor(t1[:], q2[:], cosc,
                                                    op=AluOpType.mult)
                            nc.vector.tensor_tensor(t2[:], q1[:], sinc,
                                                    op=AluOpType.mult)
                            nc.vector.tensor_tensor(r2[:], t1[:], t2[:],
                                                    op=AluOpType.add)
                            # store: rows of r1 are heads 4jp..4jp+3, d 0..31
                            r0 = ch * 8  # 8 grid rows per chunk
                            eng = nc.sync if sect == 0 else nc.scalar
                            for hh in range(4):
                                h = 4 * jp + hh
                                d0 = (h % 2) * 64
                                eng.dma_start(
                                    dram[h // 2, d0:d0 + 32, r0:r0 + 8, 0:G],
                                    r1[32 * hh:32 * (hh + 1), :]
                                    .rearrange("d (r c) -> d r c", c=G))
                                eng.dma_start(
                                    dram[h // 2, d0 + 32:d0 + 64, r0:r0 + 8, 0:G],
                                    r2[32 * hh:32 * (hh + 1), :]
                                    .rearrange("d (r c) -> d r c", c=G))
                    # v token-major
                    for t4 in range(4):
                        tok0 = ch * CH + t4 * 128
                        vbf = a_sb.tile([128, C], BF16, tag="vbf", bufs=2,
                                        name=f"vbf_{ch}_{t4}")
                        for nchunk, (f0, fn) in enumerate(((0, 512), (512, 256))):
                            vps = a_vps.tile([128, 512], F32, tag="mm512", bufs=3,
                                             name=f"vps_{ch}_{t4}_{nchunk}")
                            for c in range(CC):
                                nc.tensor.matmul(
                                    vps[:, 0:fn],
                                    xlnT[c][:, t4 * 128:(t4 + 1) * 128],
                                    wqkT[c][:, 2 * C + f0:2 * C + f0 + fn],
                                    start=(c == 0), stop=(c == CC - 1))
                            nc.vector.tensor_copy(vbf[:, f0:f0 + fn], vps[:, 0:fn])
                        # scatter rows into window scratch
                        gr0 = (ch * CH + t4 * 128) // G  # first grid row
                        for rr in range(2):
                            r = gr0 + rr
                            wr, r_in = divmod(r, WS)
                            for wc in range(NW1):
                                w = wr * NW1 + wc
                                cw = 8 if wc == NW1 - 1 else WS
                                nc.gpsimd.dma_start(
                                    v_dram[w, r_in * WS:r_in * WS + cw, :],
                                    vbf[rr * 64 + wc * WS:rr * 64 + wc * WS + cw, :])

                def do_B(w):
                    wr, wc = divmod(w, NW1)
                    r0, c0 = wr * WS, wc * WS
                    for hp in range(NH // 2):
                        qw = b_sb.tile([128, T], BF16, tag="qw",
                                       name=f"qw_{w}_{hp}")
                        kw = b_sb.tile([128, T], BF16, tag="kw",
                                       name=f"kw_{w}_{hp}")
                        nc.sync.dma_start(
                            qw[:].rearrange("d (r c) -> d r c", c=WS),
                            q_dram[hp, :, r0:r0 + WS, c0:c0 + WS])
                        nc.scalar.dma_start(
                            kw[:].rearrange("d (r c) -> d r c", c=WS),
                            k_dram[hp, :, r0:r0 + WS, c0:c0 + WS])
                        vp0 = b_sb.tile([KC0, 128], BF16, tag="v0",
                                        name=f"v0_{w}_{hp}")
                        vp1 = b_sb.tile([KC1, 128], BF16, tag="v1",
                                        name=f"v1_{w}_{hp}")
                        nc.gpsimd.dma_start(
                            vp0[:], v_dram[w, 0:KC0, hp * 128:(hp + 1) * 128])
                        nc.gpsimd.dma_start(
                            vp1[:], v_dram[w, KC0:T, hp * 128:(hp + 1) * 128])
                        ab = b_sb.tile([128, T], BF16, tag="ab", name=f"ab_{w}_{hp}")
                        for hh in range(2):
                            p0 = hh * 64
                            s0 = b_ps.tile([KC0, T], F32, tag="s", bufs=2,
                                           name=f"s0_{w}_{hp}_{hh}")
                            s1 = b_ps.tile([KC1, T], F32, tag="s", bufs=2,
                                           name=f"s1_{w}_{hp}_{hh}")
                            nc.tensor.matmul(s0[:], kw[p0:p0 + 64, 0:KC0],
                                             qw[p0:p0 + 64, :],
                                             start=True, stop=True)
                            nc.tensor.matmul(s1[:], kw[p0:p0 + 64, KC0:T],
                                             qw[p0:p0 + 64, :],
                                             start=True, stop=True)
                            e0 = b_sb.tile([KC0, T], BF16, tag="e0",
                                           name=f"e0_{w}_{hp}_{hh}")
                            e1 = b_sb.tile([KC1, T], BF16, tag="e1",
                                           name=f"e1_{w}_{hp}_{hh}")
                            nc.scalar.activation(e0[:], s0[:], AF.Exp,
                                                 scale=HD ** -0.5)
                            nc.scalar.activation(e1[:], s1[:], AF.Exp,
                                                 scale=HD ** -0.5)
                            o = b_ops.tile([HD, T], F32, tag="o", bufs=2,
                                           name=f"o_{w}_{hp}_{hh}")
                            nc.tensor.matmul(o[:], vp0[:, p0:p0 + 64], e0[:],
                                             start=True, stop=False)
                            nc.tensor.matmul(o[:], vp1[:, p0:p0 + 64], e1[:],
                                             start=False, stop=True)
                            sm = b_ops.tile([1, T], F32, tag="o", bufs=2,
                                            name=f"sm_{w}_{hp}_{hh}")
                            nc.tensor.matmul(sm[:], ones_col[0:KC0, :], e0[:],
                                             start=True, stop=False)
                            nc.tensor.matmul(sm[:], ones_col[0:KC1, :], e1[:],
                                             start=False, stop=True)
                            rcp = b_sb.tile([1, T], F32, tag="rcp",
                                            name=f"rcp_{w}_{hp}_{hh}")
                            nc.vector.reciprocal(rcp[:], sm[:])
                            rb = b_sb.tile([HD, T], F32, tag="rb",
                                           name=f"rb_{w}_{hp}_{hh}")
                            nc.gpsimd.partition_broadcast(rb[:], rcp[:])
                            nc.vector.tensor_tensor(ab[p0:p0 + 64, :], o[:], rb[:],
                                                    op=AluOpType.mult)
                        nc.gpsimd.dma_start(
                            att_dram[hp, :, r0:r0 + WS, c0:c0 + WS],
                            ab[:].rearrange("d (r c) -> d r c", c=WS))

                def do_CD(ch):
                    c_sb = _pools["c_sb"]
                    attC = []
                    gr0c = ch * 8
                    for c in range(CC):
                        atc = c_sb.tile([128, CH], BF16, tag=f"attC{c}", bufs=2,
                                        name=f"attC_{ch}_{c}")
                        nc.sync.dma_start(
                            atc[:].rearrange("f (r c) -> f r c", c=G),
                            att_dram[c, :, gr0c:gr0c + 8, 0:G])
                        attC.append(atc)
                    yT = [c_sb.tile([128, CH], BF16, tag=f"yT{c}",
                                    name=f"yT_{ch}_{c}") for c in range(CC)]
                    for t4 in range(4):
                        tok0 = ch * CH + t4 * 128
                        gr0 = tok0 // G
                        attT = [attC[c][:, t4 * 128:(t4 + 1) * 128]
                                for c in range(CC)]
                        x1 = c_sb.tile([128, C], F32, tag="x1", bufs=2,
                                       name=f"x1_{ch}_{t4}")
                        xt = c_sb.tile([128, C], F32, tag="xt2", bufs=2,
                                       name=f"xt2_{ch}_{t4}")
                        nc.sync.dma_start(xt[:], x_t[tok0:tok0 + 128, :])
                        for f0, fn in ((0, 512), (512, 256)):
                            pps = c_ps.tile([128, 512], F32, tag="mm512", bufs=3,
                                            name=f"pps_{ch}_{t4}_{f0}")
                            for c in range(CC):
                                nc.tensor.matmul(pps[:, 0:fn], attT[c],
                                                 wpT[c][:, f0:f0 + fn],
                                                 start=(c == 0), stop=(c == CC - 1))
                            nc.vector.scalar_tensor_tensor(
                                x1[:, f0:f0 + fn], pps[:, 0:fn], 1.0,
                                bp_bcast[:, f0:f0 + fn],
                                op0=AluOpType.mult, op1=AluOpType.add)
                        nc.vector.tensor_tensor(x1[:], x1[:], xt[:],
                                                op=AluOpType.add)
                        nc.sync.dma_start(x1_dram[tok0:tok0 + 128, :], x1[:])
                        # LN2
                        stats = c_sb.tile([128, 12], F32, tag="stats2", bufs=3,
                                          name=f"st2_{ch}_{t4}")
                        nc.vector.bn_stats(stats[:, 0:6], x1[:, 0:384])
                        nc.vector.bn_stats(stats[:, 6:12], x1[:, 384:768])
                        mv = c_sb.tile([128, 2], F32, tag="mv2", bufs=3,
                                       name=f"mv2_{ch}_{t4}")
                        nc.vector.bn_aggr(mv[:], stats[:])
                        rs = c_sb.tile([128, 1], F32, tag="rs2", bufs=3,
                                       name=f"rs2_{ch}_{t4}")
                        nc.vector.tensor_scalar_add(rs[:], mv[:, 1:2], LN_EPS)
                        nc.vector.reciprocal(rs[:], rs[:])
                        nc.scalar.sqrt(rs[:], rs[:])
                        y = c_sb.tile([128, C], BF16, tag="y", bufs=3,
                                      name=f"y_{ch}_{t4}")
                        nc.vector.tensor_scalar(y[:], x1[:], mv[:, 0:1], rs[:],
                                                op0=AluOpType.subtract,
                                                op1=AluOpType.mult)
                        for c in range(CC):
                            trp = c_tr_ps.tile([128, 128], BF16, tag="tr", bufs=1,
                                               name=f"trp2_{ch}_{t4}_{c}")
                            nc.tensor.transpose(trp[:], y[:, c * 128:(c + 1) * 128],
                                                ident[:])
                            nc.vector.tensor_copy(
                                yT[c][:, t4 * 128:(t4 + 1) * 128], trp[:])
                    # ============ PHASE D (per chunk): MLP ============
                    h1 = [c_sb.tile([128, CH], BF16, tag=f"h1_{j}", bufs=1,
                                    name=f"h1_{ch}_{j}") for j in range(24)]
                    for j in range(24):
                        hps = c_ps.tile([128, 512], F32, tag="mm512", bufs=3,
                                        name=f"hps_{ch}_{j}")
                        for c in range(CC):
                            nc.tensor.matmul(hps[:], w1T[c][:, j * 128:(j + 1) * 128],
                                             yT[c][:], start=(c == 0),
                                             stop=(c == CC - 1))
                        if not sim_gelu:
                            nc.scalar.activation(h1[j][:], hps[:], AF.Gelu,
                                                 bias=bias_fc1_fm[:, j:j + 1])
                        else:
                            # CoreSim lacks Gelu: tanh-approx decomposition
                            tg = c_sb.tile([128, CH], F32, tag="tg", bufs=2,
                                           name=f"tg_{ch}_{j}")
                            nc.scalar.activation(tg[:], hps[:], AF.Identity,
                                                 bias=bias_fc1_fm[:, j:j + 1])
                            sq = c_sb.tile([128, CH], F32, tag="sq", bufs=2,
                                           name=f"sq_{ch}_{j}")
                            nc.scalar.activation(sq[:], tg[:], AF.Square)
                            nc.vector.tensor_scalar(sq[:], sq[:], 0.044715, 1.0,
                                                    op0=AluOpType.mult,
                                                    op1=AluOpType.add)
                            nc.vector.tensor_tensor(sq[:], sq[:], tg[:],
                                                    op=AluOpType.mult)
                            nc.scalar.activation(sq[:], sq[:], AF.Tanh,
                                                 scale=0.7978845608028654)
                            nc.vector.tensor_scalar(sq[:], sq[:], 1.0, 0.5,
                                                    op0=AluOpType.add,
                                                    op1=AluOpType.mult)
                            nc.vector.tensor_tensor(h1[j][:], sq[:], tg[:],
                                                    op=AluOpType.mult)
                    for t4 in range(4):
                        tok0 = ch * CH + t4 * 128
                        x1t = c_sb.tile([128, C], F32, tag="x1t", bufs=2,
                                        name=f"x1t_{ch}_{t4}")
                        nc.sync.dma_start(x1t[:], x1_dram[tok0:tok0 + 128, :])
                        ot = c_sb.tile([128, C], F32, tag="ot", bufs=2,
                                       name=f"ot_{ch}_{t4}")
                        for f0, fn in ((0, 512), (512, 256)):
                            ops_ = c_ps.tile([128, 512], F32, tag="mm512", bufs=3,
                                             name=f"ops_{ch}_{t4}_{f0}")
                            for j in range(24):
                                nc.tensor.matmul(
                                    ops_[:, 0:fn],
                                    h1[j][:, t4 * 128:(t4 + 1) * 128],
                                    w2T[j][:, f0:f0 + fn],
                                    start=(j == 0), stop=(j == 23))
                            nc.vector.scalar_tensor_tensor(
                                ot[:, f0:f0 + fn], ops_[:, 0:fn], 1.0,
                                b2_bcast[:, f0:f0 + fn],
                                op0=AluOpType.mult, op1=AluOpType.add)
                        nc.vector.tensor_tensor(ot[:], ot[:], x1t[:],
                                                op=AluOpType.add)
                        nc.sync.dma_start(out_t[tok0:tok0 + 128, :], ot[:])

                # interleaved emission: window-row wr needs grid rows
                # <= wr*14+13 (A chunk ch covers rows 8ch..8ch+7); CD chunk
                # ch needs att rows <= 8ch+7 (windows through those rows).
                do_A(0); do_A(1)
                do_B(0); do_B(1); do_B(2); do_B(3); do_B(4)          # wr=0
                do_A(2); do_A(3)
                do_B(5); do_B(6); do_B(7); do_B(8); do_B(9)          # wr=1
                do_A(4); do_A(5)
                do_B(10); do_B(11); do_B(12); do_B(13); do_B(14)     # wr=2
                do_A(6); do_A(7)
                a_sb.release()
                wqk.release()
                _pools["c_sb"] = tc.alloc_tile_pool(name="c_sb", bufs=2)
                do_B(15); do_B(16); do_B(17); do_B(18); do_B(19)     # wr=3
                do_CD(0); do_CD(1); do_CD(2)
                do_B(20); do_B(21); do_B(22); do_B(23); do_B(24)     # wr=4
                do_CD(3); do_CD(4); do_CD(5); do_CD(6); do_CD(7)
                _pools["c_sb"].release()

    nc.finalize()
    return nc


def kernel(**inputs) -> np.ndarray:
    global _COMPILED
    from concourse.bass_utils import run_bass_kernel_spmd

    if _COMPILED is None:
        _COMPILED = _build()
    nc = _COMPILED

    x = np.ascontiguousarray(np.asarray(inputs["x"], dtype=np.float32))
    rope = np.ascontiguousarray(
        np.asarray(inputs["rope_2d"], dtype=np.float32).reshape(G, G, HD))
    shared = {
        k: np.ascontiguousarray(np.asarray(inputs[k], dtype=np.float32))
        for k in ("ln1_g", "ln1_b", "w_qkv", "b_qkv", "w_proj", "b_proj",
                  "gamma_1", "ln2_g", "ln2_b", "w_fc1", "b_fc1", "w_fc2",
                  "b_fc2", "gamma_2")
    }
    in_maps = [{"x": x[b], "rope_2d": rope, **shared} for b in range(8)]
    res = run_bass_kernel_spmd(nc, in_maps, list(range(8)))
    return np.stack([res.results[b]["out"] for b in range(8)]).astype(np.float32)
